# revision 1
# baseline (speedup 1.0000x reference)
"""Linear attention (silu+1 feature map) MultiHeadAttention kernel for 8x TRN2.

Sharding: data-parallel over batch (B=8 -> 1 batch element per NeuronCore).
Per-core math (T=4096, D=1024, H=16, Dh=64), all matmuls bf16 / fp32 PSUM:

  phase 1 (stream token tiles):
    qT[o,t]   = WqT.T @ xT          (feature-major, stationary = WqT chunks)
    phi_qT    = silu(s*qT + s*bq) + 1        (stored bf16, feature-major)
    k[t,e]    = xT.T @ WkT          (token-major, stationary = xT chunks)
    v[t,e]    = xT.T @ WvT + bv
    phi_k     = silu(s*k) + 1
    vk_h[e,d] += v_h.T @ phi_k_h    (PSUM accumulate, 16 heads packed in 1 bank)
  M stage:
    M_h[d,o]  = vk_h.T @ WoT_h      ->  M = vstack_h(M_h)   [1024,1024] bf16
  phase 2:
    yT[o,t]   = M.T @ phi_qT + bo   (one dense GEMM; folds per-head phi_q@kv
                                     and the output projection together)

Host side: transposes x per batch, pre-transposes/casts weights to bf16,
gathers yT.T per core. Output fp32.
"""

import numpy as np
import ml_dtypes

B, T, D = 8, 4096, 1024
H, DH = 16, 64
SCALE = float(DH ** -0.25)
NCORES = 8
P = 128
DC = D // P          # 8 feature chunks
TT = 512             # token tile (phase 1)
NTT = T // TT        # 8 token tiles
NSUB = TT // P       # 4 sub-tiles of 128 tokens

_BF16 = ml_dtypes.bfloat16

_CACHE = {}


def _split_multi_waits(nc):
    """walrus in this container only encodes ONE sync-wait command per
    instruction. Hoist extra waits onto injected same-engine NOPs placed
    immediately before the instruction (program order on the engine queue
    makes this semantically identical)."""
    import concourse.mybir as mybir

    n_split = 0
    for fn in nc.m.functions:
        for bb in fn.blocks:
            new = []
            changed = False
            for inst in bb.instructions:
                si = inst.sync_info
                waits = list(si.on_wait) if si is not None else []
                if len(waits) > 1:
                    changed = True
                    for j, w in enumerate(waits[:-1]):
                        nop = mybir.InstNoOp(
                            name=f"{inst.name}-sw{j}", ins=[], outs=[]
                        )
                        nop.engine = inst.engine
                        nop.sync_info = mybir.SyncInfo(
                            on_wait=[w], on_update=[]
                        )
                        new.append(nop)
                        n_split += 1
                    inst.sync_info = mybir.SyncInfo(
                        on_wait=[waits[-1]], on_update=list(si.on_update)
                    )
                new.append(inst)
            if changed:
                bb.instructions = new
    return n_split


def _build_program(debug=False):
    import concourse.bass as bass
    import concourse.mybir as mybir
    from concourse.tile import TileContext, add_dep_helper

    dt = mybir.dt
    AF = mybir.ActivationFunctionType

    nc = bass.Bass()

    xT_d = nc.dram_tensor("xT", [D, T], dt.bfloat16, kind="ExternalInput")
    wq_d = nc.dram_tensor("wq", [D, D], dt.bfloat16, kind="ExternalInput")
    wk_d = nc.dram_tensor("wk", [D, D], dt.bfloat16, kind="ExternalInput")
    wv_d = nc.dram_tensor("wv", [D, D], dt.bfloat16, kind="ExternalInput")
    wo_d = nc.dram_tensor("wo", [D, D], dt.bfloat16, kind="ExternalInput")
    bqs_d = nc.dram_tensor("bqs", [P, DC], dt.float32, kind="ExternalInput")
    bos_d = nc.dram_tensor("bos", [P, DC], dt.float32, kind="ExternalInput")
    bvb_d = nc.dram_tensor("bvb", [P, D], dt.float32, kind="ExternalInput")
    yT_d = nc.dram_tensor("yT", [D, T], dt.float32, kind="ExternalOutput")
    if debug:
        phiq_d = nc.dram_tensor("phiq_dump", [P, DC, T], dt.bfloat16, kind="ExternalOutput")
        kv_d = nc.dram_tensor("kv_dump", [P, 512], dt.float32, kind="ExternalOutput")
        m_d = nc.dram_tensor("m_dump", [P, DC, D], dt.bfloat16, kind="ExternalOutput")
        kproj_d = nc.dram_tensor("kproj_dump", [P, D], dt.float32, kind="ExternalOutput")
        vproj_d = nc.dram_tensor("vproj_dump", [P, D], dt.float32, kind="ExternalOutput")

    with TileContext(nc) as tc:
        with (
            tc.tile_pool(name="weights", bufs=1) as wpool,
            tc.tile_pool(name="phiq", bufs=1) as qpool,
            tc.tile_pool(name="msb", bufs=1) as mpool,
            tc.tile_pool(name="xin", bufs=3) as xpool,
            tc.tile_pool(name="kvtiles", bufs=6) as kvpool,
            tc.tile_pool(name="yout", bufs=2) as ypool,
        ):
            # ---- weight / const preload ----
            # wq + the first x pair come first (they gate the first matmuls);
            # x tiles stream on the gpsimd queue, weights on sync, wo (only
            # needed at the M stage) last.
            wq_sb = wpool.tile([P, DC, D], dt.bfloat16, tag="wq")
            wk_sb = wpool.tile([P, DC, D], dt.bfloat16, tag="wk")
            wv_sb = wpool.tile([P, DC, D], dt.bfloat16, tag="wv")
            wo_sb = wpool.tile([P, DC, D], dt.bfloat16, tag="wo")
            bq_sb = wpool.tile([P, DC], dt.float32, tag="bq")
            bo_sb = wpool.tile([P, DC], dt.float32, tag="bo")
            bv_sb = wpool.tile([P, D], dt.float32, tag="bv")
            nc.sync.dma_start(bq_sb[:], bqs_d[:])
            nc.sync.dma_start(bo_sb[:], bos_d[:])
            # wq in column halves: the first q matmuls (oc 0-3) only need the
            # first half, so PE starts ~3us earlier. bvb (0.5MB) is not needed
            # until the first kv sub-tile (~30us in), so it loads after wk.
            wq_r = wq_d.rearrange("(c p) o -> p c o", p=P)
            xT_r = xT_d.rearrange("(c p) t -> p c t", p=P)
            nc.sync.dma_start(wq_sb[:, :, 0:512], wq_r[:, :, 0:512])
            xt_pre = []
            for half in range(2):
                xt0 = xpool.tile([P, DC, TT], dt.bfloat16, tag="xt", name=f"xtpre{half}")
                nc.sync.dma_start(xt0[:], xT_r[:, :, half * TT : (half + 1) * TT])
                xt_pre.append(xt0)
            nc.sync.dma_start(wq_sb[:, :, 512:1024], wq_r[:, :, 512:1024])
            nc.sync.dma_start(wk_sb[:], wk_d.rearrange("(c p) o -> p c o", p=P))
            nc.sync.dma_start(bv_sb[:], bvb_d[:])
            nc.sync.dma_start(wv_sb[:], wv_d.rearrange("(c p) o -> p c o", p=P))
            nc.sync.dma_start(wo_sb[:], wo_d.rearrange("(c p) o -> p c o", p=P))

            phi_q = qpool.tile([P, DC, T], dt.bfloat16, tag="phiq")
            m_chunks = []
            for c in range(DC):
                m_chunk = mpool.tile(
                    [P, D], dt.bfloat16, tag=f"msb{c}", name=f"msb{c}"
                )
                m_chunks.append(m_chunk)
            kv_chunks = []
            for c in range(DC):
                kvc = mpool.tile(
                    [P, P], dt.bfloat16, tag=f"kvsb{c}", name=f"kvsb{c}"
                )
                kv_chunks.append(kvc)

            zz = wpool.tile([1, 640], dt.bfloat16, tag="zz")
            nc.vector.memset(zz[:], 0.0)
            # kv chunk off-diagonal blocks must be zero (block-diag repack)
            for c in range(DC):
                nc.vector.memset(kv_chunks[c][:], 0.0)

            with tc.tile_pool(name="ps_kv", bufs=1, space="PSUM") as pkv_pool:
                kv_ps = pkv_pool.tile([P, 512], dt.float32, tag="kvacc")
                # zero the whole kv bank once (sets has_written for every
                # element) so the 16 interleaved head slots can accumulate
                # with start=False; multiple start=True groups in one bank
                # clobber each other.
                nc.tensor.matmul(
                    kv_ps[:], lhsT=zz[:1, :P], rhs=zz[:1, P : P + 512],
                    start=True, stop=True, skip_group_check=True,
                )
                # warmup matmuls filling the startup DMA shadow: semantically
                # they re-write zeros over the (unused-yet) kv bank, but they
                # keep the PE p-state/HAM warm so the first real matmuls run
                # at full clock the moment weights land. N=128 keeps the
                # granularity fine so the last one barely delays real work.
                for w in range(88):
                    nc.tensor.matmul(
                        kv_ps[:, 0:128], lhsT=zz[:1, :P], rhs=zz[:1, P : P + 128],
                        start=True, stop=True, skip_group_check=True,
                    )
                nc.tensor.matmul(
                    kv_ps[:], lhsT=zz[:1, :P], rhs=zz[:1, P : P + 512],
                    start=True, stop=True, skip_group_check=True,
                )

                kv_pend = [None]

                def _emit_kv(pending, last):
                    phik_p, vsb_p = pending
                    for h in range(H):
                        r0 = (h % 2) * 64
                        c0 = (h // 2) * 64
                        nc.tensor.matmul(
                            kv_ps[r0 : r0 + 64, c0 : c0 + 64],
                            lhsT=vsb_p[:, h * 64 : (h + 1) * 64],
                            rhs=phik_p[:, h * 64 : (h + 1) * 64],
                            start=False,
                            stop=last and h == H - 1,
                            skip_group_check=True,
                        )

                with tc.tile_pool(name="ps_q", bufs=3, space="PSUM") as pq_pool:
                  with tc.tile_pool(name="ps_kvp", bufs=2, space="PSUM") as pkvp_pool:
                      # ---- q projection (feature-major out); both tiles of a
                      # pair share each stationary weight load. first_split runs
                      # tile A before tile B (pair 0: B's DMA still in flight).
                      # flush_after_oc0 emits the last kv matmuls between q
                      # chunks so the M stage overlaps the q stream.
                      def _q_section(pair, xts, first_split, post_oc=None):
                          def _drain(oc, half, psx):
                              tt = pair * 2 + half
                              pq_slice = phi_q[:, oc, tt * TT : (tt + 1) * TT]
                              nc.scalar.activation(
                                  pq_slice, psx[:], AF.Silu,
                                  bias=bq_sb[:, oc : oc + 1], scale=SCALE,
                              )
                              nc.vector.tensor_scalar_add(pq_slice, pq_slice, 1.0)

                          for oc in range(DC):
                              psA = pq_pool.tile([P, TT], dt.float32, tag="psq")
                              psB = pq_pool.tile([P, TT], dt.float32, tag="psq")
                              if first_split and oc == 0:
                                  last_a = None
                                  for d in range(DC):
                                      last_a = nc.tensor.matmul(
                                          psA[:],
                                          lhsT=wq_sb[:, d, oc * P : (oc + 1) * P],
                                          rhs=xts[0][:, d, :],
                                          start=(d == 0),
                                          stop=(d == DC - 1),
                                      )
                                  # bridge the B-tile DMA wait with warmup
                                  # zero-rewrites of the (still unused) kv bank;
                                  # dep-pinned after the A matmuls so the
                                  # scheduler cannot hoist them earlier
                                  for w in range(14):
                                      dmy = nc.tensor.matmul(
                                          kv_ps[:, 0:128],
                                          lhsT=zz[:1, :P],
                                          rhs=zz[:1, P : P + 128],
                                          start=True, stop=True,
                                          skip_group_check=True,
                                      )
                                      add_dep_helper(
                                          dmy.ins, last_a.ins, sync=False,
                                          reason="bridge dummies after A matmuls",
                                      )
                                  dmy = nc.tensor.matmul(
                                      kv_ps[:], lhsT=zz[:1, :P],
                                      rhs=zz[:1, P : P + 512],
                                      start=True, stop=True,
                                      skip_group_check=True,
                                  )
                                  add_dep_helper(
                                      dmy.ins, last_a.ins, sync=False,
                                      reason="bridge dummies after A matmuls",
                                  )
                                  for d in range(DC):
                                      nc.tensor.matmul(
                                          psB[:],
                                          lhsT=wq_sb[:, d, oc * P : (oc + 1) * P],
                                          rhs=xts[1][:, d, :],
                                          start=(d == 0),
                                          stop=(d == DC - 1),
                                      )
                              else:
                                  for d in range(DC):
                                      nc.tensor.matmul(
                                          psA[:],
                                          lhsT=wq_sb[:, d, oc * P : (oc + 1) * P],
                                          rhs=xts[0][:, d, :],
                                          start=(d == 0),
                                          stop=(d == DC - 1),
                                      )
                                      nc.tensor.matmul(
                                          psB[:],
                                          lhsT=wq_sb[:, d, oc * P : (oc + 1) * P],
                                          rhs=xts[1][:, d, :],
                                          start=(d == 0),
                                          stop=(d == DC - 1),
                                      )
                              _drain(oc, 0, psA)
                              _drain(oc, 1, psB)
                              if post_oc is not None and oc in post_oc:
                                  post_oc[oc]()

                      # ---- k,v projections (token-major) + kv accumulation.
                      # The 16 kv-accumulate matmuls for a sub-tile are emitted
                      # one sub-tile LATE so PE never waits on silu/+bv. ----
                      def _kvproj_section(pair, xts):
                          for half in range(2):
                              tt = pair * 2 + half
                              xt = xts[half]
                              for sub in range(NSUB):
                                  pk = pkvp_pool.tile([P, D], dt.float32, tag="pkv")
                                  pv = pkvp_pool.tile([P, D], dt.float32, tag="pkv")
                                  xs = xt[:, :, sub * P : (sub + 1) * P]
                                  for d in range(DC):
                                      for n in range(2):
                                          nc.tensor.matmul(
                                              pk[:, n * 512 : (n + 1) * 512],
                                              lhsT=xs[:, d, :],
                                              rhs=wk_sb[:, d, n * 512 : (n + 1) * 512],
                                              start=(d == 0),
                                              stop=(d == DC - 1),
                                          )
                                      for n in range(2):
                                          nc.tensor.matmul(
                                              pv[:, n * 512 : (n + 1) * 512],
                                              lhsT=xs[:, d, :],
                                              rhs=wv_sb[:, d, n * 512 : (n + 1) * 512],
                                              start=(d == 0),
                                              stop=(d == DC - 1),
                                          )
                                  if debug and tt == 0 and sub == 0:
                                      kpf = mpool.tile([P, D], dt.float32, tag="kpdump")
                                      vpf = mpool.tile([P, D], dt.float32, tag="vpdump")
                                      nc.vector.tensor_copy(out=kpf[:], in_=pk[:])
                                      nc.vector.tensor_copy(out=vpf[:], in_=pv[:])
                                      nc.sync.dma_start(kproj_d[:], kpf[:])
                                      nc.sync.dma_start(vproj_d[:], vpf[:])
                                  phik = kvpool.tile([P, D], dt.bfloat16, tag="phik")
                                  vsb = kvpool.tile([P, D], dt.bfloat16, tag="vsb")
                                  nc.scalar.activation(
                                      phik[:], pk[:], AF.Silu, scale=SCALE
                                  )
                                  nc.vector.tensor_scalar_add(phik[:], phik[:], 1.0)
                                  nc.vector.tensor_add(vsb[:], pv[:], bv_sb[:])
                                  if kv_pend[0] is not None:
                                      _emit_kv(kv_pend[0], False)
                                  kv_pend[0] = (phik, vsb)

                      for pair in range(NTT // 2):
                          if pair == 0:
                              xts = xt_pre
                          else:
                              xts = []
                              for half in range(2):
                                  tt = pair * 2 + half
                                  xt = xpool.tile([P, DC, TT], dt.bfloat16, tag="xt")
                                  nc.gpsimd.dma_start(
                                      xt[:], xT_r[:, :, tt * TT : (tt + 1) * TT]
                                  )
                                  xts.append(xt)

                          if pair == NTT // 2 - 1:
                              # last pair: kvproj first, then q. The kv flush,
                              # repack copies, M matmuls (psum borrowed from the
                              # drained kvproj pool — no extra banks) and
                              # m-chunk copies are spread across the q chunk
                              # boundaries, fully hidden under the 27us of q
                              # matmuls with no engine head-of-line blocking.
                              _kvproj_section(pair, xts)

                              def _hook_flush():
                                  _emit_kv(kv_pend[0], True)
                                  kv_pend[0] = None
                                  for c in range(DC):
                                      if c % 2 == 0:
                                          nc.vector.tensor_copy(
                                              out=kv_chunks[c][0:64, 0:64],
                                              in_=kv_ps[0:64, c * 64 : (c + 1) * 64],
                                          )
                                          nc.vector.tensor_copy(
                                              out=kv_chunks[c][64:128, 64:128],
                                              in_=kv_ps[64:128, c * 64 : (c + 1) * 64],
                                          )
                                      else:
                                          nc.scalar.copy(
                                              out=kv_chunks[c][0:64, 0:64],
                                              in_=kv_ps[0:64, c * 64 : (c + 1) * 64],
                                          )
                                          nc.scalar.copy(
                                              out=kv_chunks[c][64:128, 64:128],
                                              in_=kv_ps[64:128, c * 64 : (c + 1) * 64],
                                          )

                              def _mk_hook_m(c0):
                                  def _hook():
                                      for c in (c0, c0 + 1):
                                          pm = pkvp_pool.tile(
                                              [P, D], dt.float32, tag="pkv"
                                          )
                                          for n in range(2):
                                              nc.tensor.matmul(
                                                  pm[:, n * 512 : (n + 1) * 512],
                                                  lhsT=kv_chunks[c][:],
                                                  rhs=wo_sb[:, c, n * 512 : (n + 1) * 512],
                                                  start=True,
                                                  stop=True,
                                              )
                                          nc.vector.tensor_copy(
                                              out=m_chunks[c][:, 0:512],
                                              in_=pm[:, 0:512],
                                          )
                                          nc.scalar.copy(
                                              out=m_chunks[c][:, 512:1024],
                                              in_=pm[:, 512:1024],
                                          )
                                  return _hook

                              hooks = {0: _hook_flush}
                              for c0 in range(0, DC, 2):
                                  hooks[1 + c0 // 2] = _mk_hook_m(c0)
                              _q_section(pair, xts, False, hooks)
                          else:
                              _q_section(pair, xts, pair == 0)
                              _kvproj_section(pair, xts)

                      if kv_pend[0] is not None:
                          _emit_kv(kv_pend[0], True)
                          kv_pend[0] = None

                  if debug:
                      kvf = mpool.tile([P, 512], dt.float32, tag="kvdump")
                      nc.vector.tensor_copy(out=kvf[:], in_=kv_ps[:])
                      nc.sync.dma_start(kv_d[:], kvf[:])
                  # ---- phase 2: yT = M.T @ phi_q + bo ----
                  # [128,1024] psum tiles (bufs=4): the whole-tile RAW window is 16
                  # matmuls, so each tile's drain overlaps the next tiles' matmuls
                  # and the kernel tail is just one small tile's drain.
                  with tc.tile_pool(name="ps_y", bufs=2, space="PSUM") as py_pool:
                      for oc in range(DC):
                          for qb in range(4):
                              if oc == DC - 1 and qb == 3:
                                  # very last block: two independent [128,512] psum
                                  # tiles so the final drain is one small piece that
                                  # starts 8 matmuls before the end
                                  for i in range(2):
                                      pyf = py_pool.tile([P, 512], dt.float32, tag="py")
                                      for j in range(DC):
                                          f = (oc * 4 + qb + j) % DC
                                          nc.tensor.matmul(
                                              pyf[:],
                                              lhsT=m_chunks[f][:, oc * P : (oc + 1) * P],
                                              rhs=phi_q[
                                                  :, f, qb * 1024 + i * 512 : qb * 1024 + (i + 1) * 512
                                              ],
                                              start=(j == 0),
                                              stop=(j == DC - 1),
                                          )
                                      ysf = ypool.tile(
                                          [P, 512], dt.float32, tag=f"ys{i}"
                                      )
                                      if i == 0:
                                          nc.scalar.activation(
                                              ysf[:], pyf[:], AF.Identity,
                                              bias=bo_sb[:, oc : oc + 1], scale=1.0,
                                          )
                                          nc.sync.dma_start(
                                              yT_d[
                                                  oc * P : (oc + 1) * P,
                                                  qb * 1024 : qb * 1024 + 512,
                                              ],
                                              ysf[:],
                                          )
                                      else:
                                          nc.vector.tensor_scalar_add(
                                              ysf[:], pyf[:], bo_sb[:, oc : oc + 1]
                                          )
                                          nc.gpsimd.dma_start(
                                              yT_d[
                                                  oc * P : (oc + 1) * P,
                                                  qb * 1024 + 512 : (qb + 1) * 1024,
                                              ],
                                              ysf[:],
                                          )
                                  continue
                              py = py_pool.tile([P, 1024], dt.float32, tag="py")
                              # rotated f-order: successive tiles start on different
                              # M chunks, so phase 2 begins as soon as the first
                              # chunk copy lands and the rest overlap these matmuls
                              for j in range(DC):
                                  f = (oc * 4 + qb + j) % DC
                                  for i in range(2):
                                      nc.tensor.matmul(
                                          py[:, i * 512 : (i + 1) * 512],
                                          lhsT=m_chunks[f][:, oc * P : (oc + 1) * P],
                                          rhs=phi_q[
                                              :, f, qb * 1024 + i * 512 : qb * 1024 + (i + 1) * 512
                                          ],
                                          start=(j == 0),
                                          stop=(j == DC - 1),
                                      )
                              # drain in two 512 pieces on ACT+sync / DVE+gpsimd
                              ys0 = ypool.tile([P, 512], dt.float32, tag="ys0")
                              nc.scalar.activation(
                                  ys0[:], py[:, 0:512],
                                  AF.Identity, bias=bo_sb[:, oc : oc + 1], scale=1.0,
                              )
                              nc.sync.dma_start(
                                  yT_d[
                                      oc * P : (oc + 1) * P,
                                      qb * 1024 : qb * 1024 + 512,
                                  ],
                                  ys0[:],
                              )
                              ys1 = ypool.tile([P, 512], dt.float32, tag="ys1")
                              nc.vector.tensor_scalar_add(
                                  ys1[:], py[:, 512:1024], bo_sb[:, oc : oc + 1]
                              )
                              nc.gpsimd.dma_start(
                                  yT_d[
                                      oc * P : (oc + 1) * P,
                                      qb * 1024 + 512 : (qb + 1) * 1024,
                                  ],
                                  ys1[:],
                              )

            if debug:
                nc.sync.dma_start(phiq_d[:], phi_q[:])
                for c in range(DC):
                    nc.sync.dma_start(m_d[:, c, :], m_chunks[c][:])
    _split_multi_waits(nc)
    return nc


def _get_program(debug=False):
    key = ("nc", debug)
    if key not in _CACHE:
        _CACHE[key] = _build_program(debug)
    return _CACHE[key]


def _prep_shared(Wq, bq, Wk, Wv, bv, Wo, bo):
    shared = {
        "wq": np.ascontiguousarray(Wq.T).astype(_BF16),
        "wk": np.ascontiguousarray(Wk.T).astype(_BF16),
        "wv": np.ascontiguousarray(Wv.T).astype(_BF16),
        "wo": np.ascontiguousarray(Wo.T).astype(_BF16),
        "bqs": np.ascontiguousarray(
            (SCALE * bq).astype(np.float32).reshape(DC, P).T
        ),
        "bos": np.ascontiguousarray(bo.astype(np.float32).reshape(DC, P).T),
        "bvb": np.ascontiguousarray(
            np.broadcast_to(bv.astype(np.float32), (P, D))
        ),
    }
    return shared


def _run(in_maps, trace=False, debug=False, **kw):
    from concourse.bass_utils import run_bass_kernel_spmd

    nc = _get_program(debug)
    return run_bass_kernel_spmd(nc, in_maps, list(range(NCORES)), trace=trace, **kw)


def kernel(x, Wq, bq, Wk, Wv, bv, Wo, bo):
    x = np.asarray(x, dtype=np.float32)
    assert x.shape == (B, T, D), x.shape
    shared = _prep_shared(
        np.asarray(Wq, np.float32), np.asarray(bq, np.float32),
        np.asarray(Wk, np.float32), np.asarray(Wv, np.float32),
        np.asarray(bv, np.float32), np.asarray(Wo, np.float32),
        np.asarray(bo, np.float32),
    )
    in_maps = []
    for b in range(B):
        m = dict(shared)
        m["xT"] = np.ascontiguousarray(x[b].T).astype(_BF16)
        in_maps.append(m)

    res = _run(in_maps)
    out = np.empty((B, T, D), np.float32)
    for b in range(B):
        out[b] = res.results[b]["yT"].T
    return out



# revision 9
# speedup vs baseline: 2.1212x; 2.1212x over previous
"""Linear attention (silu+1 feature map) MultiHeadAttention for 8x TRN2.

Sharding: data-parallel over batch (B=8 -> 1 batch element per NeuronCore).

Math per core (T=4096, D=1024, H=16, Dh=64), with phi(z) = 1 + s(z),
s(z) = silu(z). Write s_q = silu(scale*q), s_k = silu(scale*k). Then

  kv_h   = phi_k_h^T v_h
         = colsum_v_h                      (rank-1 in e; exact, bf16/fp32)
         + (s_k^T x)_h @ Wv_h^T           (fp8 "G path": replaces v proj)
         + bv_h (x) rowsum(s_k)_h         (rank-1 correction)
  M      = kv^T-blocks @ Wo^T             (block-diag, bf16)
  y^T    = M8^T @ s_q + colsum_M + bo     (fp8; the +1 of phi_q is folded
                                           into colsum_M = ones^T M)

All big GEMMs (q proj, k proj, G = s_k^T x, phase-2) run as fp8-e4m3
DoubleRow matmuls (2x128-row contraction @ 0.5 cyc/row).  Centering the
+1 out of phi keeps fp8 noise confined to the ~12%-magnitude fluctuation
terms; exact colsums are carried in fp32/bf16.  Weights are scaled by 64
before fp8 quantization to clear the e4m3 subnormal floor; the inverse
scale rides the ACT silu drain.
"""

import numpy as np
import ml_dtypes

B, T, D = 8, 4096, 1024
H, DH = 16, 64
SCALE = float(DH ** -0.25)
NCORES = 8
P = 128
DC = D // P            # 8 feature chunks
NG = T // 256          # 16 groups of 256 tokens
WS = 64.0              # fp8 weight prescale

_BF16 = ml_dtypes.bfloat16
_F8 = ml_dtypes.float8_e4m3

_CACHE = {}


def _split_multi_waits(nc):
    """walrus in this container only encodes ONE sync-wait command per
    instruction. Hoist extra waits onto injected same-engine NOPs placed
    immediately before the instruction."""
    import concourse.mybir as mybir

    n_split = 0
    for fn in nc.m.functions:
        for bb in fn.blocks:
            new = []
            changed = False
            for inst in bb.instructions:
                si = inst.sync_info
                waits = list(si.on_wait) if si is not None else []
                if len(waits) > 1:
                    changed = True
                    for j, w in enumerate(waits[:-1]):
                        nop = mybir.InstNoOp(
                            name=f"{inst.name}-sw{j}", ins=[], outs=[]
                        )
                        nop.engine = inst.engine
                        nop.sync_info = mybir.SyncInfo(
                            on_wait=[w], on_update=[]
                        )
                        new.append(nop)
                        n_split += 1
                    inst.sync_info = mybir.SyncInfo(
                        on_wait=[waits[-1]], on_update=list(si.on_update)
                    )
                new.append(inst)
            if changed:
                bb.instructions = new
    return n_split


def _build_program(debug=False):
    import concourse.bass as bass
    import concourse.mybir as mybir
    from concourse.tile import TileContext

    dt = mybir.dt
    AF = mybir.ActivationFunctionType
    DR = mybir.MatmulPerfMode.DoubleRow

    nc = bass.Bass()

    xT8_d = nc.dram_tensor("xT8", [P, DC, T], dt.float8e4, kind="ExternalInput")
    xtb_d = nc.dram_tensor("xtb", [P, DC, T], dt.bfloat16, kind="ExternalInput")
    xt8_d = nc.dram_tensor("xt8", [P, NG, 2, D], dt.float8e4, kind="ExternalInput")
    wq8_d = nc.dram_tensor("wq8", [P, DC, D], dt.float8e4, kind="ExternalInput")
    wk8_d = nc.dram_tensor("wk8", [P, DC, D], dt.float8e4, kind="ExternalInput")
    wvT_d = nc.dram_tensor("wvT", [P, DC, D], dt.bfloat16, kind="ExternalInput")
    woT_d = nc.dram_tensor("woT", [P, DC, D], dt.bfloat16, kind="ExternalInput")
    bqb_d = nc.dram_tensor("bqb", [P, DC, 256], dt.float32, kind="ExternalInput")
    bvc_d = nc.dram_tensor("bvc", [P, DC], dt.float32, kind="ExternalInput")
    bob_d = nc.dram_tensor("bob", [P, DC], dt.float32, kind="ExternalInput")
    bvr_d = nc.dram_tensor("bvr", [1, D], dt.bfloat16, kind="ExternalInput")
    one8_d = nc.dram_tensor("one8", [P, 2, 16], dt.float8e4, kind="ExternalInput")
    yT_d = nc.dram_tensor("yT", [P, DC, T], dt.bfloat16, kind="ExternalOutput")
    if debug:
        dbg = {
            "sq": nc.dram_tensor("dbg_sq", [P, DC, T], dt.float8e4, kind="ExternalOutput"),
            "sk": nc.dram_tensor("dbg_sk", [P, NG, 2, D], dt.float8e4, kind="ExternalOutput"),
            "gt": nc.dram_tensor("dbg_gt", [P, DC, D], dt.bfloat16, kind="ExternalOutput"),
            "kv": nc.dram_tensor("dbg_kv", [P, DC, P], dt.bfloat16, kind="ExternalOutput"),
            "m8": nc.dram_tensor("dbg_m8", [P, DC, D], dt.float8e4, kind="ExternalOutput"),
            "csx": nc.dram_tensor("dbg_csx", [P, DC], dt.float32, kind="ExternalOutput"),
            "cv": nc.dram_tensor("dbg_cv", [P, DC], dt.float32, kind="ExternalOutput"),
            "rs": nc.dram_tensor("dbg_rs", [1, D], dt.bfloat16, kind="ExternalOutput"),
            "by": nc.dram_tensor("dbg_by", [P, DC], dt.float32, kind="ExternalOutput"),
        }

    with TileContext(nc) as tc:
        with tc.tile_pool(name="persist", bufs=1) as pp:
            bqb_sb = pp.tile([P, DC, 256], dt.float32, tag="bqb")
            bvc_sb = pp.tile([P, DC], dt.float32, tag="bvc")
            bob_sb = pp.tile([P, DC], dt.float32, tag="bob")
            bvr_sb = pp.tile([1, D], dt.bfloat16, tag="bvr")
            one8_sb = pp.tile([P, 2, 16], dt.float8e4, tag="one8")
            zz = pp.tile([1, 640], dt.bfloat16, tag="zz")
            csx_sb = pp.tile([P, DC], dt.float32, tag="csx")
            csxb_sb = pp.tile([P, DC], dt.bfloat16, tag="csxb")
            cv_sb = pp.tile([P, DC], dt.float32, tag="cv")
            u_sb = pp.tile([P, DC], dt.float32, tag="u")
            ub_sb = pp.tile([P, DC], dt.bfloat16, tag="ub")
            by_sb = pp.tile([P, DC], dt.float32, tag="by")
            rs_sb = pp.tile([1, D], dt.bfloat16, tag="rs")
            kvch = pp.tile([P, DC, P], dt.bfloat16, tag="kvch")
            m8_sb = pp.tile([P, DC, D], dt.float8e4, tag="m8")
            sq_sb = pp.tile([P, DC, T], dt.float8e4, tag="sq")

            nc.vector.memset(zz[:], 0.0)
            nc.vector.memset(kvch[:], 0.0)

            with tc.tile_pool(name="bigB", bufs=1) as pb:
                sk_sb = pb.tile([P, NG, 2, D], dt.float8e4, tag="sk")
                xt8_sb = pb.tile([P, NG, 2, D], dt.float8e4, tag="xt8")

                # ---------------- phase 1: q/k projections ----------------
                with (
                    tc.tile_pool(name="ph1w", bufs=1) as pc,
                    tc.tile_pool(name="xtbp", bufs=2) as pxtb,
                ):
                    wq8_sb = pc.tile([P, DC, D], dt.float8e4, tag="wq8")
                    wk8_sb = pc.tile([P, DC, D], dt.float8e4, tag="wk8")
                    xT8_sb = pc.tile([P, DC, T], dt.float8e4, tag="xT8")

                    # one queue (gpsimd: 25ns/trigger), strict priority order
                    nc.gpsimd.dma_start(wq8_sb[:], wq8_d[:])
                    nc.gpsimd.dma_start(bqb_sb[:], bqb_d[:])
                    nc.gpsimd.dma_start(xT8_sb[:, :, 0:1024], xT8_d[:, :, 0:1024])
                    nc.gpsimd.dma_start(wk8_sb[:], wk8_d[:])
                    nc.gpsimd.dma_start(xT8_sb[:, :, 1024:2048], xT8_d[:, :, 1024:2048])
                    nc.gpsimd.dma_start(xT8_sb[:, :, 2048:3072], xT8_d[:, :, 2048:3072])
                    nc.gpsimd.dma_start(xT8_sb[:, :, 3072:4096], xT8_d[:, :, 3072:4096])
                    nc.gpsimd.dma_start(one8_sb[:], one8_d[:])
                    nc.gpsimd.dma_start(bvc_sb[:], bvc_d[:])
                    nc.gpsimd.dma_start(bob_sb[:], bob_d[:])
                    nc.gpsimd.dma_start(bvr_sb[:], bvr_d[:])
                    for qq in range(4):
                        nc.gpsimd.dma_start(
                            xt8_sb[:, qq * 4 : (qq + 1) * 4, :, :],
                            xt8_d[:, qq * 4 : (qq + 1) * 4, :, :],
                        )

                    # colsum_x: streamed bf16 eighths of xT, DVE reduced
                    xtb_tiles = []
                    for e in range(8):
                        xtbt = pxtb.tile([P, DC, 512], dt.bfloat16, tag="xtbt")
                        nc.gpsimd.dma_start(
                            xtbt[:], xtb_d[:, :, e * 512 : (e + 1) * 512]
                        )
                        xtb_tiles.append(xtbt)

                    csx_emitted = [0]

                    def _emit_csx_step():
                        e = csx_emitted[0]
                        if e >= 8:
                            return
                        csx_emitted[0] += 1
                        if e == 0:
                            nc.vector.tensor_reduce(
                                csx_sb[:], xtb_tiles[e][:],
                                axis=mybir.AxisListType.X, op=mybir.AluOpType.add,
                            )
                        else:
                            tmp = pxtb.tile([P, DC], dt.float32, tag="csxt")
                            nc.vector.tensor_reduce(
                                tmp[:], xtb_tiles[e][:],
                                axis=mybir.AxisListType.X, op=mybir.AluOpType.add,
                            )
                            nc.vector.tensor_add(csx_sb[:], csx_sb[:], tmp[:])

                    with (
                        tc.tile_pool(name="qps", bufs=2, space="PSUM") as qpool,
                        tc.tile_pool(name="kps", bufs=2, space="PSUM") as kpool,
                    ):
                        # warmup matmuls: keep PE busy during initial DMA
                        warm = qpool.tile([P, 4, 256], dt.float32, tag="qp")
                        for w in range(110):
                            nc.tensor.matmul(
                                warm[:, 0, 0:128], lhsT=zz[:1, :P],
                                rhs=zz[:1, P : P + 128],
                                start=True, stop=True, skip_group_check=True,
                            )

                        def _q_group(g):
                            for oh in range(2):
                                qp = qpool.tile([P, 4, 256], dt.float32, tag="qp")
                                for j in range(4):
                                    oc = oh * 4 + j
                                    for c in range(4):
                                        nc.tensor.matmul(
                                            qp[:, j, :],
                                            lhsT=wq8_sb[:, 2 * c : 2 * c + 2, oc * P : (oc + 1) * P],
                                            rhs=xT8_sb[:, 2 * c : 2 * c + 2, g * 256 : (g + 1) * 256],
                                            start=(c == 0), stop=(c == 3),
                                            perf_mode=DR, skip_group_check=True,
                                        )
                                # bias add on DVE, silu drain on ACT
                                nc.vector.tensor_add(
                                    qp[:], qp[:],
                                    bqb_sb[:, oh * 4 : (oh + 1) * 4, :],
                                )
                                nc.scalar.activation(
                                    sq_sb[:, oh * 4 : (oh + 1) * 4, g * 256 : (g + 1) * 256],
                                    qp[:], AF.Silu, scale=SCALE / WS,
                                )

                        def _k_group(g):
                            for i in range(2):
                                kp = kpool.tile([P, D], dt.float32, tag="kp")
                                t0 = g * 256 + i * 128
                                for ds in range(4):
                                    for c in range(4):
                                        nc.tensor.matmul(
                                            kp[:, ds * 256 : (ds + 1) * 256],
                                            lhsT=xT8_sb[:, 2 * c : 2 * c + 2, t0 : t0 + 128],
                                            rhs=wk8_sb[:, 2 * c : 2 * c + 2, ds * 256 : (ds + 1) * 256],
                                            start=(c == 0), stop=(c == 3),
                                            perf_mode=DR, skip_group_check=True,
                                        )
                                nc.scalar.activation(
                                    sk_sb[:, g, i, :], kp[:], AF.Silu,
                                    scale=SCALE / WS,
                                )

                        # order: q 0-3 first (only needs wq8+xT8h0); k tails
                        # 3 q-groups before the end so ACT clears s_k early
                        for g in range(4):
                            _q_group(g)
                            _emit_csx_step()
                        for g in range(4, 12):
                            _k_group(g - 4)
                            _q_group(g)
                            _emit_csx_step()
                        for g in range(8, 13):
                            _k_group(g)
                        _q_group(12)
                        _emit_csx_step()
                        for g in range(13, 16):
                            _k_group(g)
                        for g in range(13, 16):
                            _q_group(g)
                            _emit_csx_step()
                        while csx_emitted[0] < 8:
                            _emit_csx_step()
                        nc.vector.tensor_copy(out=csxb_sb[:], in_=csx_sb[:])

                # ---------------- G / rs / cv / kv / M / cm ----------------
                with tc.tile_pool(name="postw", bufs=1) as pd:
                    wvT_sb = pd.tile([P, DC, D], dt.bfloat16, tag="wvT")
                    woT_sb = pd.tile([P, DC, D], dt.bfloat16, tag="woT")
                    gt_sb = pd.tile([P, DC, D], dt.bfloat16, tag="gt")
                    nc.sync.dma_start(wvT_sb[:], wvT_d[:])
                    nc.sync.dma_start(woT_sb[:], woT_d[:])

                    with (
                        tc.tile_pool(name="gps", bufs=2, space="PSUM") as gpool,
                        tc.tile_pool(name="rsps", bufs=1, space="PSUM") as rspool,
                    ):
                        rsp = rspool.tile([1, D], dt.float32, tag="rsp")
                        cvp = rspool.tile([P, DC], dt.float32, tag="cvp")

                        def _g_chunk(cc):
                            gp = gpool.tile([P, D], dt.float32, tag="gp")
                            # pre-zero the whole tile so the 4 interleaved
                            # 256-col regions can accumulate start=False
                            for hh in range(2):
                                nc.tensor.matmul(
                                    gp[:, hh * 512 : (hh + 1) * 512],
                                    lhsT=zz[:1, :P], rhs=zz[:1, P : P + 512],
                                    start=True, stop=True, skip_group_check=True,
                                )
                            for g in range(NG):
                                for ds in range(4):
                                    nc.tensor.matmul(
                                        gp[:, ds * 256 : (ds + 1) * 256],
                                        lhsT=xt8_sb[:, g, :, cc * P : (cc + 1) * P],
                                        rhs=sk_sb[:, g, :, ds * 256 : (ds + 1) * 256],
                                        start=False, stop=(g == NG - 1),
                                        perf_mode=DR, skip_group_check=True,
                                    )
                            nc.scalar.copy(out=gt_sb[:, cc, :], in_=gp[:])

                        _g_chunk(0)
                        _g_chunk(1)
                        # rowsum(s_k): ones^T s_k  -> [1, D]
                        for ds in range(4):
                            for g in range(NG):
                                nc.tensor.matmul(
                                    rsp[:, ds * 256 : (ds + 1) * 256],
                                    lhsT=one8_sb[:, :, 0:1],
                                    rhs=sk_sb[:, g, :, ds * 256 : (ds + 1) * 256],
                                    start=(g == 0), stop=(g == NG - 1),
                                    perf_mode=DR, skip_group_check=True,
                                )
                        nc.scalar.copy(out=rs_sb[:], in_=rsp[:])
                        for cc in range(2, DC):
                            _g_chunk(cc)
                        # colsum_v = Wv @ colsum_x  (+ T*bv via bvc)
                        for b in range(DC):
                            for cc in range(DC):
                                nc.tensor.matmul(
                                    cvp[:, b : b + 1],
                                    lhsT=wvT_sb[:, cc, b * P : (b + 1) * P],
                                    rhs=csxb_sb[:, cc : cc + 1],
                                    start=(cc == 0), stop=(cc == DC - 1),
                                    skip_group_check=True,
                                )
                        nc.vector.tensor_add(cv_sb[:], cvp[:], bvc_sb[:])

                    # kv blocks
                    with tc.tile_pool(name="kvps", bufs=2, space="PSUM") as kvpool:
                        for half in range(2):
                            kvp = kvpool.tile([P, 4, P], dt.float32, tag="kvp")
                            for j in range(4):
                                b = half * 4 + j
                                for cc in range(DC):
                                    nc.tensor.matmul(
                                        kvp[:, j, :],
                                        lhsT=wvT_sb[:, cc, b * P : (b + 1) * P],
                                        rhs=gt_sb[:, cc, b * P : (b + 1) * P],
                                        start=(cc == 0), stop=False,
                                        skip_group_check=True,
                                    )
                                nc.tensor.matmul(
                                    kvp[:, j, :],
                                    lhsT=bvr_sb[:1, b * P : (b + 1) * P],
                                    rhs=rs_sb[:1, b * P : (b + 1) * P],
                                    start=False, stop=True, skip_group_check=True,
                                )
                            for j in range(4):
                                b = half * 4 + j
                                nc.scalar.activation(
                                    kvch[0:64, b, 0:64], kvp[0:64, j, 0:64],
                                    AF.Identity, bias=cv_sb[0:64, b : b + 1],
                                )
                                nc.scalar.activation(
                                    kvch[64:128, b, 64:128], kvp[64:128, j, 64:128],
                                    AF.Identity, bias=cv_sb[64:128, b : b + 1],
                                )
                                nc.vector.tensor_reduce(
                                    u_sb[0:64, b : b + 1], kvch[0:64, b, 0:64],
                                    axis=mybir.AxisListType.X, op=mybir.AluOpType.add,
                                )
                                nc.vector.tensor_reduce(
                                    u_sb[64:128, b : b + 1], kvch[64:128, b, 64:128],
                                    axis=mybir.AxisListType.X, op=mybir.AluOpType.add,
                                )
                        nc.vector.tensor_copy(out=ub_sb[:], in_=u_sb[:])

                    # M = kv^T @ Wo^T ; colsum_M
                    with tc.tile_pool(name="mps", bufs=2, space="PSUM") as mpool:
                        for b in range(DC):
                            mp = mpool.tile([P, D], dt.float32, tag="mp")
                            for hh in range(2):
                                nc.tensor.matmul(
                                    mp[:, hh * 512 : (hh + 1) * 512],
                                    lhsT=kvch[:, b, :],
                                    rhs=woT_sb[:, b, hh * 512 : (hh + 1) * 512],
                                    start=True, stop=True, skip_group_check=True,
                                )
                            nc.scalar.copy(out=m8_sb[:, b, :], in_=mp[:])
                        cmp_t = mpool.tile([P, DC], dt.float32, tag="cmp")
                        for oc in range(DC):
                            for b in range(DC):
                                nc.tensor.matmul(
                                    cmp_t[:, oc : oc + 1],
                                    lhsT=woT_sb[:, b, oc * P : (oc + 1) * P],
                                    rhs=ub_sb[:, b : b + 1],
                                    start=(b == 0), stop=(b == DC - 1),
                                    skip_group_check=True,
                                )
                        nc.vector.tensor_add(by_sb[:], cmp_t[:], bob_sb[:])

                    if debug:
                        nc.sync.dma_start(dbg["sq"][:], sq_sb[:])
                        nc.sync.dma_start(dbg["sk"][:], sk_sb[:])
                        nc.sync.dma_start(dbg["gt"][:], gt_sb[:])
                        nc.sync.dma_start(dbg["kv"][:], kvch[:])
                        nc.sync.dma_start(dbg["m8"][:], m8_sb[:])
                        nc.sync.dma_start(dbg["csx"][:], csx_sb[:])
                        nc.sync.dma_start(dbg["cv"][:], cv_sb[:])
                        nc.sync.dma_start(dbg["rs"][:], rs_sb[:])
                        nc.sync.dma_start(dbg["by"][:], by_sb[:])

            # ---------------- phase 2: y^T = M8^T s_q + bias ----------------
            with (
                tc.tile_pool(name="yout", bufs=4) as ypool,
                tc.tile_pool(name="yps", bufs=4, space="PSUM") as ypsp,
            ):
                n = 0
                for oc in range(DC):
                    for tp in range(8):
                        yp = ypsp.tile([P, 512], dt.float32, tag="yp")
                        for hh in range(2):
                            ts = tp * 2 + hh
                            for f in range(4):
                                nc.tensor.matmul(
                                    yp[:, hh * 256 : (hh + 1) * 256],
                                    lhsT=m8_sb[:, 2 * f : 2 * f + 2, oc * P : (oc + 1) * P],
                                    rhs=sq_sb[:, 2 * f : 2 * f + 2, ts * 256 : (ts + 1) * 256],
                                    start=(f == 0), stop=(f == 3),
                                    perf_mode=DR, skip_group_check=True,
                                )
                        ys = ypool.tile([P, 512], dt.bfloat16, tag="ys")
                        if n % 2 == 0:
                            nc.scalar.activation(
                                ys[:], yp[:], AF.Identity,
                                bias=by_sb[:, oc : oc + 1], scale=1.0,
                            )
                        else:
                            nc.vector.tensor_scalar_add(
                                ys[:], yp[:], by_sb[:, oc : oc + 1]
                            )
                        (nc.sync if n % 2 == 0 else nc.gpsimd).dma_start(
                            yT_d[:, oc, tp * 512 : (tp + 1) * 512], ys[:]
                        )
                        n += 1

    _split_multi_waits(nc)
    return nc


def _get_program(debug=False):
    key = ("nc", debug)
    if key not in _CACHE:
        _CACHE[key] = _build_program(debug)
    return _CACHE[key]


def _prep_shared(Wq, bq, Wk, Wv, bv, Wo, bo):
    def wchunk(w, dtype, scale=1.0):
        # [D, D] row-major (contract, out) -> [P, DC, D] with c = cc*128+p
        return np.ascontiguousarray(
            (w * scale).T.reshape(DC, P, D).transpose(1, 0, 2)
        ).astype(dtype)

    # DVE pre-adds bqb to the (WS-scaled) q PSUM; ACT then multiplies the
    # sum by SCALE/WS, so the bias must carry WS (not SCALE) here.
    bqs = (WS * bq).astype(np.float32).reshape(DC, P).T  # [P, DC]
    shared = {
        "wq8": wchunk(Wq, _F8, WS),
        "wk8": wchunk(Wk, _F8, WS),
        "wvT": wchunk(Wv, _BF16),
        "woT": wchunk(Wo, _BF16),
        "bqb": np.ascontiguousarray(
            np.broadcast_to(bqs[:, :, None], (P, DC, 256))
        ).astype(np.float32),
        "bvc": np.ascontiguousarray((T * bv).astype(np.float32).reshape(DC, P).T),
        "bob": np.ascontiguousarray(bo.astype(np.float32).reshape(DC, P).T),
        "bvr": bv.astype(_BF16)[None, :],
        "one8": np.ones((P, 2, 16), _F8),
    }
    return shared


def _prep_x(xb):
    xT = np.ascontiguousarray(xb.T)  # [D, T]
    return {
        "xT8": np.ascontiguousarray(
            xT.reshape(DC, P, T).transpose(1, 0, 2)
        ).astype(_F8),
        "xtb": np.ascontiguousarray(
            xT.reshape(DC, P, T).transpose(1, 0, 2)
        ).astype(_BF16),
        "xt8": np.ascontiguousarray(
            xb.reshape(NG, 2, P, D).transpose(2, 0, 1, 3)
        ).astype(_F8),
    }


def _run(in_maps, trace=False, debug=False, **kw):
    from concourse.bass_utils import run_bass_kernel_spmd

    nc = _get_program(debug)
    return run_bass_kernel_spmd(nc, in_maps, list(range(len(in_maps))), trace=trace, **kw)


def kernel(x, Wq, bq, Wk, Wv, bv, Wo, bo):
    x = np.asarray(x, dtype=np.float32)
    assert x.shape == (B, T, D), x.shape
    shared = _prep_shared(
        np.asarray(Wq, np.float32), np.asarray(bq, np.float32),
        np.asarray(Wk, np.float32), np.asarray(Wv, np.float32),
        np.asarray(bv, np.float32), np.asarray(Wo, np.float32),
        np.asarray(bo, np.float32),
    )
    in_maps = []
    for b in range(B):
        m = dict(shared)
        m.update(_prep_x(x[b]))
        in_maps.append(m)

    res = _run(in_maps)
    out = np.empty((B, T, D), np.float32)
    for b in range(B):
        yT = np.asarray(res.results[b]["yT"]).astype(np.float32)  # [P, DC, T]
        out[b] = yT.transpose(1, 0, 2).reshape(D, T).T
    return out


# revision 12
# speedup vs baseline: 2.5980x; 1.2247x over previous
"""Linear attention (silu+1 feature map) MultiHeadAttention for 8x TRN2.

Sharding: data-parallel over batch (B=8 -> 1 batch element per NeuronCore).

Math per core (T=4096, D=1024, H=16, Dh=64), with phi(z) = 1 + s(z),
s(z) = silu(z). Write s_q = silu(scale*q), s_k = silu(scale*k). Then

  kv_h   = phi_k_h^T v_h
         = colsum_v_h                      (rank-1 in e; exact, bf16/fp32)
         + (s_k^T x)_h @ Wv_h^T           (fp8 "G path": replaces v proj)
         + bv_h (x) rowsum(s_k)_h         (rank-1 correction)
  M      = kv^T-blocks @ Wo^T             (block-diag, bf16)
  y^T    = M8^T @ s_q + colsum_M + bo     (fp8; the +1 of phi_q is folded
                                           into colsum_M = ones^T M)

All big GEMMs (q proj, k proj, G = s_k^T x, phase-2) run as fp8-e4m3
DoubleRow matmuls (2x128-row contraction @ 0.5 cyc/row).  Centering the
+1 out of phi keeps fp8 noise confined to the ~12%-magnitude fluctuation
terms; exact colsums are carried in fp32/bf16.  Weights are scaled by 64
before fp8 quantization to clear the e4m3 subnormal floor; the inverse
scale rides the ACT silu drain.
"""

import numpy as np
import ml_dtypes

B, T, D = 8, 4096, 1024
H, DH = 16, 64
SCALE = float(DH ** -0.25)
NCORES = 8
P = 128
DC = D // P            # 8 feature chunks
NG = T // 256          # 16 groups of 256 tokens
WS = 64.0              # fp8 weight prescale

_BF16 = ml_dtypes.bfloat16
_F8 = ml_dtypes.float8_e4m3

_CACHE = {}


def _split_multi_waits(nc):
    """walrus in this container only encodes ONE sync-wait command per
    instruction. Hoist extra waits onto injected same-engine NOPs placed
    immediately before the instruction."""
    import concourse.mybir as mybir

    n_split = 0
    for fn in nc.m.functions:
        for bb in fn.blocks:
            new = []
            changed = False
            for inst in bb.instructions:
                si = inst.sync_info
                waits = list(si.on_wait) if si is not None else []
                if len(waits) > 1:
                    changed = True
                    for j, w in enumerate(waits[:-1]):
                        nop = mybir.InstNoOp(
                            name=f"{inst.name}-sw{j}", ins=[], outs=[]
                        )
                        nop.engine = inst.engine
                        nop.sync_info = mybir.SyncInfo(
                            on_wait=[w], on_update=[]
                        )
                        new.append(nop)
                        n_split += 1
                    inst.sync_info = mybir.SyncInfo(
                        on_wait=[waits[-1]], on_update=list(si.on_update)
                    )
                new.append(inst)
            if changed:
                bb.instructions = new
    return n_split


def _build_program(debug=False):
    import concourse.bass as bass
    import concourse.mybir as mybir
    from concourse.tile import TileContext

    dt = mybir.dt
    AF = mybir.ActivationFunctionType
    DR = mybir.MatmulPerfMode.DoubleRow

    nc = bass.Bass()

    xT8_d = nc.dram_tensor("xT8", [P, DC, T], dt.float8e4, kind="ExternalInput")
    xtb_d = nc.dram_tensor("xtb", [P, DC, T], dt.bfloat16, kind="ExternalInput")
    xt8_d = nc.dram_tensor("xt8", [P, NG, 2, D], dt.float8e4, kind="ExternalInput")
    wq8_d = nc.dram_tensor("wq8", [P, DC, D], dt.float8e4, kind="ExternalInput")
    wk8_d = nc.dram_tensor("wk8", [P, DC, D], dt.float8e4, kind="ExternalInput")
    wvT_d = nc.dram_tensor("wvT", [P, DC, D], dt.bfloat16, kind="ExternalInput")
    woT_d = nc.dram_tensor("woT", [P, DC, D], dt.bfloat16, kind="ExternalInput")
    bqb_d = nc.dram_tensor("bqb", [P, DC, 256], dt.float32, kind="ExternalInput")
    bvc_d = nc.dram_tensor("bvc", [P, DC], dt.float32, kind="ExternalInput")
    bob_d = nc.dram_tensor("bob", [P, DC], dt.float32, kind="ExternalInput")
    bvr_d = nc.dram_tensor("bvr", [1, D], dt.bfloat16, kind="ExternalInput")
    one8_d = nc.dram_tensor("one8", [P, 2, 16], dt.float8e4, kind="ExternalInput")
    yT_d = nc.dram_tensor("yT", [P, DC, T], dt.bfloat16, kind="ExternalOutput")
    if debug:
        dbg = {
            "sq": nc.dram_tensor("dbg_sq", [P, DC, T], dt.float8e4, kind="ExternalOutput"),
            "sk": nc.dram_tensor("dbg_sk", [P, NG, 2, D], dt.float8e4, kind="ExternalOutput"),
            "gt": nc.dram_tensor("dbg_gt", [P, DC, D], dt.bfloat16, kind="ExternalOutput"),
            "kv": nc.dram_tensor("dbg_kv", [P, DC, P], dt.bfloat16, kind="ExternalOutput"),
            "m8": nc.dram_tensor("dbg_m8", [P, DC, D], dt.float8e4, kind="ExternalOutput"),
            "csx": nc.dram_tensor("dbg_csx", [P, DC], dt.float32, kind="ExternalOutput"),
            "cv": nc.dram_tensor("dbg_cv", [P, DC], dt.float32, kind="ExternalOutput"),
            "rs": nc.dram_tensor("dbg_rs", [1, D], dt.bfloat16, kind="ExternalOutput"),
            "by": nc.dram_tensor("dbg_by", [P, DC], dt.float32, kind="ExternalOutput"),
        }

    with TileContext(nc) as tc:
        with tc.tile_pool(name="persist", bufs=1) as pp:
            bqb_sb = pp.tile([P, DC, 256], dt.float32, tag="bqb")
            bvc_sb = pp.tile([P, DC], dt.float32, tag="bvc")
            bob_sb = pp.tile([P, DC], dt.float32, tag="bob")
            bvr_sb = pp.tile([1, D], dt.bfloat16, tag="bvr")
            one8_sb = pp.tile([P, 2, 16], dt.float8e4, tag="one8")
            zz = pp.tile([1, 640], dt.bfloat16, tag="zz")
            csx_sb = pp.tile([P, DC], dt.float32, tag="csx")
            csxb_sb = pp.tile([P, DC], dt.bfloat16, tag="csxb")
            cv_sb = pp.tile([P, DC], dt.float32, tag="cv")
            u_sb = pp.tile([P, DC], dt.float32, tag="u")
            ub_sb = pp.tile([P, DC], dt.bfloat16, tag="ub")
            by_sb = pp.tile([P, DC], dt.float32, tag="by")
            rs_sb = pp.tile([1, D], dt.bfloat16, tag="rs")
            kvch = pp.tile([P, DC, P], dt.bfloat16, tag="kvch")
            m8_sb = pp.tile([P, DC, D], dt.float8e4, tag="m8")
            sq_sb = pp.tile([P, DC, T], dt.float8e4, tag="sq")

            nc.vector.memset(zz[:], 0.0)
            nc.vector.memset(kvch[:], 0.0)

            with tc.tile_pool(name="bigB", bufs=1) as pb:
                sk_sb = pb.tile([P, NG, 2, D], dt.float8e4, tag="sk")
                xt8_sb = pb.tile([P, NG, 2, D], dt.float8e4, tag="xt8")
                gt_sb = pb.tile([P, DC, D], dt.bfloat16, tag="gt")

                # ---------------- phase 1: q/k projections + G ----------------
                with (
                    tc.tile_pool(name="ph1w", bufs=1) as pc,
                    tc.tile_pool(name="xtbp", bufs=2) as pxtb,
                ):
                    wq8_sb = pc.tile([P, DC, D], dt.float8e4, tag="wq8")
                    wk8_sb = pc.tile([P, DC, D], dt.float8e4, tag="wk8")
                    xT8_sb = pc.tile([P, DC, T], dt.float8e4, tag="xT8")

                    # one queue (gpsimd: 25ns/trigger), strict priority order:
                    # 1a needs wk8+xT8 first; xt8 by ~38us (G in 1b); xtb
                    # eighths feed the DVE colsum (e0-2 reduced in 1a).
                    nc.gpsimd.dma_start(wk8_sb[:], wk8_d[:])
                    nc.gpsimd.dma_start(xT8_sb[:, :, 0:1024], xT8_d[:, :, 0:1024])
                    nc.gpsimd.dma_start(xT8_sb[:, :, 1024:2048], xT8_d[:, :, 1024:2048])
                    nc.gpsimd.dma_start(wq8_sb[:], wq8_d[:])
                    nc.gpsimd.dma_start(xT8_sb[:, :, 2048:3072], xT8_d[:, :, 2048:3072])
                    nc.gpsimd.dma_start(xT8_sb[:, :, 3072:4096], xT8_d[:, :, 3072:4096])
                    nc.gpsimd.dma_start(bqb_sb[:], bqb_d[:])
                    nc.gpsimd.dma_start(one8_sb[:], one8_d[:])
                    nc.gpsimd.dma_start(bvc_sb[:], bvc_d[:])
                    nc.gpsimd.dma_start(bob_sb[:], bob_d[:])
                    nc.gpsimd.dma_start(bvr_sb[:], bvr_d[:])

                    xtb_tiles = []

                    def _xtb_dma(e):
                        xtbt = pxtb.tile([P, DC, 512], dt.bfloat16, tag="xtbt")
                        nc.gpsimd.dma_start(
                            xtbt[:], xtb_d[:, :, e * 512 : (e + 1) * 512]
                        )
                        xtb_tiles.append(xtbt)

                    _xtb_dma(0)
                    _xtb_dma(1)
                    nc.gpsimd.dma_start(xt8_sb[:, 0:4, :, :], xt8_d[:, 0:4, :, :])
                    _xtb_dma(2)
                    nc.gpsimd.dma_start(xt8_sb[:, 4:8, :, :], xt8_d[:, 4:8, :, :])
                    nc.gpsimd.dma_start(xt8_sb[:, 8:12, :, :], xt8_d[:, 8:12, :, :])
                    nc.gpsimd.dma_start(xt8_sb[:, 12:16, :, :], xt8_d[:, 12:16, :, :])
                    for e in range(3, 8):
                        _xtb_dma(e)

                    csx_emitted = [0]

                    def _emit_csx_step():
                        e = csx_emitted[0]
                        if e >= 8:
                            return
                        csx_emitted[0] += 1
                        if e == 0:
                            nc.vector.tensor_reduce(
                                csx_sb[:], xtb_tiles[e][:],
                                axis=mybir.AxisListType.X, op=mybir.AluOpType.add,
                            )
                        else:
                            tmp = pxtb.tile([P, DC], dt.float32, tag="csxt")
                            nc.vector.tensor_reduce(
                                tmp[:], xtb_tiles[e][:],
                                axis=mybir.AxisListType.X, op=mybir.AluOpType.add,
                            )
                            nc.vector.tensor_add(csx_sb[:], csx_sb[:], tmp[:])

                    # ---- phase 1a: k projection only (deep kps pipeline) ----
                    with tc.tile_pool(name="kps", bufs=4, space="PSUM") as kpool:
                        warm = kpool.tile([P, D], dt.float32, tag="kp")
                        for w in range(80):
                            nc.tensor.matmul(
                                warm[:, 0:128], lhsT=zz[:1, :P],
                                rhs=zz[:1, P : P + 128],
                                start=True, stop=True, skip_group_check=True,
                            )

                        def _k_group(g):
                            for i in range(2):
                                kp = kpool.tile([P, D], dt.float32, tag="kp")
                                t0 = g * 256 + i * 128
                                for ds in range(4):
                                    for c in range(4):
                                        nc.tensor.matmul(
                                            kp[:, ds * 256 : (ds + 1) * 256],
                                            lhsT=xT8_sb[:, 2 * c : 2 * c + 2, t0 : t0 + 128],
                                            rhs=wk8_sb[:, 2 * c : 2 * c + 2, ds * 256 : (ds + 1) * 256],
                                            start=(c == 0), stop=(c == 3),
                                            perf_mode=DR, skip_group_check=True,
                                        )
                                nc.scalar.activation(
                                    sk_sb[:, g, i, :], kp[:], AF.Silu,
                                    scale=SCALE / WS,
                                )

                        for g in range(NG):
                            _k_group(g)
                            if g in (7, 11, 15):
                                _emit_csx_step()

                    # ---- phase 1b: q proj interleaved with G half-chunks ----
                    with (
                        tc.tile_pool(name="qps", bufs=2, space="PSUM") as qpool,
                        tc.tile_pool(name="gps", bufs=2, space="PSUM") as gpool,
                    ):
                        def _q_group(g):
                            for oh in range(2):
                                qp = qpool.tile([P, 4, 256], dt.float32, tag="qp")
                                for j in range(4):
                                    oc = oh * 4 + j
                                    for c in range(4):
                                        nc.tensor.matmul(
                                            qp[:, j, :],
                                            lhsT=wq8_sb[:, 2 * c : 2 * c + 2, oc * P : (oc + 1) * P],
                                            rhs=xT8_sb[:, 2 * c : 2 * c + 2, g * 256 : (g + 1) * 256],
                                            start=(c == 0), stop=(c == 3),
                                            perf_mode=DR, skip_group_check=True,
                                        )
                                # bias add on DVE, silu drain on ACT
                                nc.vector.tensor_add(
                                    qp[:], qp[:],
                                    bqb_sb[:, oh * 4 : (oh + 1) * 4, :],
                                )
                                nc.scalar.activation(
                                    sq_sb[:, oh * 4 : (oh + 1) * 4, g * 256 : (g + 1) * 256],
                                    qp[:], AF.Silu, scale=SCALE / WS,
                                )

                        g_tiles = {}

                        def _g_half(idx):
                            cc, half = idx // 2, idx % 2
                            if half == 0:
                                gp = gpool.tile([P, D], dt.float32, tag="gp")
                                g_tiles[cc] = gp
                                # pre-zero so interleaved 256-col regions can
                                # accumulate start=False
                                for hh in range(2):
                                    nc.tensor.matmul(
                                        gp[:, hh * 512 : (hh + 1) * 512],
                                        lhsT=zz[:1, :P], rhs=zz[:1, P : P + 512],
                                        start=True, stop=True, skip_group_check=True,
                                    )
                            else:
                                gp = g_tiles[cc]
                            for g in range(half * 8, half * 8 + 8):
                                for ds in range(4):
                                    nc.tensor.matmul(
                                        gp[:, ds * 256 : (ds + 1) * 256],
                                        lhsT=xt8_sb[:, g, :, cc * P : (cc + 1) * P],
                                        rhs=sk_sb[:, g, :, ds * 256 : (ds + 1) * 256],
                                        start=False, stop=(g == NG - 1),
                                        perf_mode=DR, skip_group_check=True,
                                    )
                            if half == 1:
                                nc.scalar.copy(out=gt_sb[:, cc, :], in_=gp[:])

                        for g in range(NG):
                            _q_group(g)
                            _g_half(g)
                            if g >= 4:
                                _emit_csx_step()
                        while csx_emitted[0] < 8:
                            _emit_csx_step()
                        nc.vector.tensor_copy(out=csxb_sb[:], in_=csx_sb[:])

                # ---------------- rs / cv / kv / M / cm ----------------
                with tc.tile_pool(name="postw", bufs=1) as pd:
                    wvT_sb = pd.tile([P, DC, D], dt.bfloat16, tag="wvT")
                    woT_sb = pd.tile([P, DC, D], dt.bfloat16, tag="woT")
                    nc.sync.dma_start(wvT_sb[:], wvT_d[:])
                    nc.sync.dma_start(woT_sb[:], woT_d[:])

                    with tc.tile_pool(name="rscv", bufs=1, space="PSUM") as rcpool:
                        rsp = rcpool.tile([1, D], dt.float32, tag="rsp")
                        cvp = rcpool.tile([P, DC], dt.float32, tag="cvp")
                        # rowsum(s_k): ones^T s_k  -> [1, D]
                        for ds in range(4):
                            for g in range(NG):
                                nc.tensor.matmul(
                                    rsp[:, ds * 256 : (ds + 1) * 256],
                                    lhsT=one8_sb[:, :, 0:1],
                                    rhs=sk_sb[:, g, :, ds * 256 : (ds + 1) * 256],
                                    start=(g == 0), stop=(g == NG - 1),
                                    perf_mode=DR, skip_group_check=True,
                                )
                        nc.scalar.copy(out=rs_sb[:], in_=rsp[:])
                        # colsum_v = Wv @ colsum_x  (+ T*bv via bvc)
                        for b in range(DC):
                            for cc in range(DC):
                                nc.tensor.matmul(
                                    cvp[:, b : b + 1],
                                    lhsT=wvT_sb[:, cc, b * P : (b + 1) * P],
                                    rhs=csxb_sb[:, cc : cc + 1],
                                    start=(cc == 0), stop=(cc == DC - 1),
                                    skip_group_check=True,
                                )
                        nc.vector.tensor_add(cv_sb[:], cvp[:], bvc_sb[:])

                    # kv blocks
                    with tc.tile_pool(name="kvps", bufs=2, space="PSUM") as kvpool:
                        for half in range(2):
                            kvp = kvpool.tile([P, 4, P], dt.float32, tag="kvp")
                            for j in range(4):
                                b = half * 4 + j
                                for cc in range(DC):
                                    nc.tensor.matmul(
                                        kvp[:, j, :],
                                        lhsT=wvT_sb[:, cc, b * P : (b + 1) * P],
                                        rhs=gt_sb[:, cc, b * P : (b + 1) * P],
                                        start=(cc == 0), stop=False,
                                        skip_group_check=True,
                                    )
                                nc.tensor.matmul(
                                    kvp[:, j, :],
                                    lhsT=bvr_sb[:1, b * P : (b + 1) * P],
                                    rhs=rs_sb[:1, b * P : (b + 1) * P],
                                    start=False, stop=True, skip_group_check=True,
                                )
                            for j in range(4):
                                b = half * 4 + j
                                nc.scalar.activation(
                                    kvch[0:64, b, 0:64], kvp[0:64, j, 0:64],
                                    AF.Identity, bias=cv_sb[0:64, b : b + 1],
                                )
                                nc.scalar.activation(
                                    kvch[64:128, b, 64:128], kvp[64:128, j, 64:128],
                                    AF.Identity, bias=cv_sb[64:128, b : b + 1],
                                )
                                nc.vector.tensor_reduce(
                                    u_sb[0:64, b : b + 1], kvch[0:64, b, 0:64],
                                    axis=mybir.AxisListType.X, op=mybir.AluOpType.add,
                                )
                                nc.vector.tensor_reduce(
                                    u_sb[64:128, b : b + 1], kvch[64:128, b, 64:128],
                                    axis=mybir.AxisListType.X, op=mybir.AluOpType.add,
                                )
                        nc.vector.tensor_copy(out=ub_sb[:], in_=u_sb[:])

                    # M = kv^T @ Wo^T ; colsum_M
                    with tc.tile_pool(name="mps", bufs=2, space="PSUM") as mpool:
                        for b in range(DC):
                            mp = mpool.tile([P, D], dt.float32, tag="mp")
                            for hh in range(2):
                                nc.tensor.matmul(
                                    mp[:, hh * 512 : (hh + 1) * 512],
                                    lhsT=kvch[:, b, :],
                                    rhs=woT_sb[:, b, hh * 512 : (hh + 1) * 512],
                                    start=True, stop=True, skip_group_check=True,
                                )
                            if b % 2 == 0:
                                nc.scalar.copy(out=m8_sb[:, b, :], in_=mp[:])
                            else:
                                nc.vector.tensor_copy(out=m8_sb[:, b, :], in_=mp[:])
                        cmp_t = mpool.tile([P, DC], dt.float32, tag="cmp")
                        for oc in range(DC):
                            for b in range(DC):
                                nc.tensor.matmul(
                                    cmp_t[:, oc : oc + 1],
                                    lhsT=woT_sb[:, b, oc * P : (oc + 1) * P],
                                    rhs=ub_sb[:, b : b + 1],
                                    start=(b == 0), stop=(b == DC - 1),
                                    skip_group_check=True,
                                )
                        nc.vector.tensor_add(by_sb[:], cmp_t[:], bob_sb[:])

                    if debug:
                        nc.sync.dma_start(dbg["sq"][:], sq_sb[:])
                        nc.sync.dma_start(dbg["sk"][:], sk_sb[:])
                        nc.sync.dma_start(dbg["gt"][:], gt_sb[:])
                        nc.sync.dma_start(dbg["kv"][:], kvch[:])
                        nc.sync.dma_start(dbg["m8"][:], m8_sb[:])
                        nc.sync.dma_start(dbg["csx"][:], csx_sb[:])
                        nc.sync.dma_start(dbg["cv"][:], cv_sb[:])
                        nc.sync.dma_start(dbg["rs"][:], rs_sb[:])
                        nc.sync.dma_start(dbg["by"][:], by_sb[:])

            # ---------------- phase 2: y^T = M8^T s_q + bias ----------------
            with (
                tc.tile_pool(name="yout", bufs=12) as ypool,
                tc.tile_pool(name="yps", bufs=8, space="PSUM") as ypsp,
            ):
                n = 0
                for oc in range(DC):
                    for tp in range(8):
                        yp = ypsp.tile([P, 512], dt.float32, tag="yp")
                        for hh in range(2):
                            ts = tp * 2 + hh
                            for f in range(4):
                                nc.tensor.matmul(
                                    yp[:, hh * 256 : (hh + 1) * 256],
                                    lhsT=m8_sb[:, 2 * f : 2 * f + 2, oc * P : (oc + 1) * P],
                                    rhs=sq_sb[:, 2 * f : 2 * f + 2, ts * 256 : (ts + 1) * 256],
                                    start=(f == 0), stop=(f == 3),
                                    perf_mode=DR, skip_group_check=True,
                                )
                        ys = ypool.tile([P, 512], dt.bfloat16, tag="ys")
                        if n % 2 == 0:
                            nc.scalar.activation(
                                ys[:], yp[:], AF.Identity,
                                bias=by_sb[:, oc : oc + 1], scale=1.0,
                            )
                        else:
                            nc.vector.tensor_scalar_add(
                                ys[:], yp[:], by_sb[:, oc : oc + 1]
                            )
                        (nc.sync if n % 2 == 0 else nc.gpsimd).dma_start(
                            yT_d[:, oc, tp * 512 : (tp + 1) * 512], ys[:]
                        )
                        n += 1

    _split_multi_waits(nc)
    return nc


def _get_program(debug=False):
    key = ("nc", debug)
    if key not in _CACHE:
        _CACHE[key] = _build_program(debug)
    return _CACHE[key]


def _prep_shared(Wq, bq, Wk, Wv, bv, Wo, bo):
    def wchunk(w, dtype, scale=1.0):
        # [D, D] row-major (contract, out) -> [P, DC, D] with c = cc*128+p
        return np.ascontiguousarray(
            (w * scale).T.reshape(DC, P, D).transpose(1, 0, 2)
        ).astype(dtype)

    # DVE pre-adds bqb to the (WS-scaled) q PSUM; ACT then multiplies the
    # sum by SCALE/WS, so the bias must carry WS (not SCALE) here.
    bqs = (WS * bq).astype(np.float32).reshape(DC, P).T  # [P, DC]
    shared = {
        "wq8": wchunk(Wq, _F8, WS),
        "wk8": wchunk(Wk, _F8, WS),
        "wvT": wchunk(Wv, _BF16),
        "woT": wchunk(Wo, _BF16),
        "bqb": np.ascontiguousarray(
            np.broadcast_to(bqs[:, :, None], (P, DC, 256))
        ).astype(np.float32),
        "bvc": np.ascontiguousarray((T * bv).astype(np.float32).reshape(DC, P).T),
        "bob": np.ascontiguousarray(bo.astype(np.float32).reshape(DC, P).T),
        "bvr": bv.astype(_BF16)[None, :],
        "one8": np.ones((P, 2, 16), _F8),
    }
    return shared


def _prep_x(xb):
    xT = np.ascontiguousarray(xb.T)  # [D, T]
    return {
        "xT8": np.ascontiguousarray(
            xT.reshape(DC, P, T).transpose(1, 0, 2)
        ).astype(_F8),
        "xtb": np.ascontiguousarray(
            xT.reshape(DC, P, T).transpose(1, 0, 2)
        ).astype(_BF16),
        "xt8": np.ascontiguousarray(
            xb.reshape(NG, 2, P, D).transpose(2, 0, 1, 3)
        ).astype(_F8),
    }


def _run(in_maps, trace=False, debug=False, **kw):
    from concourse.bass_utils import run_bass_kernel_spmd

    nc = _get_program(debug)
    return run_bass_kernel_spmd(nc, in_maps, list(range(len(in_maps))), trace=trace, **kw)


def kernel(x, Wq, bq, Wk, Wv, bv, Wo, bo):
    x = np.asarray(x, dtype=np.float32)
    assert x.shape == (B, T, D), x.shape
    shared = _prep_shared(
        np.asarray(Wq, np.float32), np.asarray(bq, np.float32),
        np.asarray(Wk, np.float32), np.asarray(Wv, np.float32),
        np.asarray(bv, np.float32), np.asarray(Wo, np.float32),
        np.asarray(bo, np.float32),
    )
    in_maps = []
    for b in range(B):
        m = dict(shared)
        m.update(_prep_x(x[b]))
        in_maps.append(m)

    res = _run(in_maps)
    out = np.empty((B, T, D), np.float32)
    for b in range(B):
        yT = np.asarray(res.results[b]["yT"]).astype(np.float32)  # [P, DC, T]
        out[b] = yT.transpose(1, 0, 2).reshape(D, T).T
    return out


# revision 16
# speedup vs baseline: 2.8313x; 1.0898x over previous
"""Linear attention (silu+1 feature map) MultiHeadAttention for 8x TRN2.

Sharding: data-parallel over batch (B=8 -> 1 batch element per NeuronCore).

Math per core (T=4096, D=1024, H=16, Dh=64), with phi(z) = 1 + s(z),
s(z) = silu(z). Write s_q = silu(scale*q), s_k = silu(scale*k). Then

  kv_h   = phi_k_h^T v_h
         = colsum_v_h                      (rank-1 in e; exact, bf16/fp32)
         + (s_k^T x)_h @ Wv_h^T           (fp8 "G path": replaces v proj)
         + bv_h (x) rowsum(s_k)_h         (rank-1 correction)
  M      = kv^T-blocks @ Wo^T             (block-diag, bf16)
  y^T    = M8^T @ s_q + colsum_M + bo     (fp8; the +1 of phi_q is folded
                                           into colsum_M = ones^T M)

All big GEMMs (q proj, k proj, G = s_k^T x, phase-2) run as fp8-e4m3
DoubleRow matmuls (2x128-row contraction @ 0.5 cyc/row).  Centering the
+1 out of phi keeps fp8 noise confined to the ~12%-magnitude fluctuation
terms; exact colsums are carried in fp32/bf16.  Weights are scaled by 64
before fp8 quantization to clear the e4m3 subnormal floor; the inverse
scale rides the ACT silu drain.
"""

import numpy as np
import ml_dtypes

B, T, D = 8, 4096, 1024
H, DH = 16, 64
SCALE = float(DH ** -0.25)
NCORES = 8
P = 128
DC = D // P            # 8 feature chunks
NG = T // 256          # 16 groups of 256 tokens
WS = 64.0              # fp8 weight prescale

_BF16 = ml_dtypes.bfloat16
_F8 = ml_dtypes.float8_e4m3

_CACHE = {}


def _split_multi_waits(nc):
    """walrus in this container only encodes ONE sync-wait command per
    instruction. Hoist extra waits onto injected same-engine NOPs placed
    immediately before the instruction."""
    import concourse.mybir as mybir

    n_split = 0
    for fn in nc.m.functions:
        for bb in fn.blocks:
            new = []
            changed = False
            for inst in bb.instructions:
                si = inst.sync_info
                waits = list(si.on_wait) if si is not None else []
                if len(waits) > 1:
                    changed = True
                    for j, w in enumerate(waits[:-1]):
                        nop = mybir.InstNoOp(
                            name=f"{inst.name}-sw{j}", ins=[], outs=[]
                        )
                        nop.engine = inst.engine
                        nop.sync_info = mybir.SyncInfo(
                            on_wait=[w], on_update=[]
                        )
                        new.append(nop)
                        n_split += 1
                    inst.sync_info = mybir.SyncInfo(
                        on_wait=[waits[-1]], on_update=list(si.on_update)
                    )
                new.append(inst)
            if changed:
                bb.instructions = new
    return n_split


def _build_program(debug=False):
    import concourse.bass as bass
    import concourse.mybir as mybir
    from concourse.tile import TileContext

    dt = mybir.dt
    AF = mybir.ActivationFunctionType
    DR = mybir.MatmulPerfMode.DoubleRow

    nc = bass.Bass()

    xT8_d = nc.dram_tensor("xT8", [P, DC, T], dt.float8e4, kind="ExternalInput")
    xt8_d = nc.dram_tensor("xt8", [P, NG, 2, D], dt.float8e4, kind="ExternalInput")
    xr8_d = nc.dram_tensor("xr8", [P, NG, 2, D], dt.float8e4, kind="ExternalInput")
    wq8_d = nc.dram_tensor("wq8", [P, DC, D], dt.float8e4, kind="ExternalInput")
    wk8_d = nc.dram_tensor("wk8", [P, DC, D], dt.float8e4, kind="ExternalInput")
    wvT_d = nc.dram_tensor("wvT", [P, DC, D], dt.bfloat16, kind="ExternalInput")
    woT_d = nc.dram_tensor("woT", [P, DC, D], dt.bfloat16, kind="ExternalInput")
    bqs_d = nc.dram_tensor("bqs", [P, DC], dt.float32, kind="ExternalInput")
    bqa_d = nc.dram_tensor("bqa", [P, DC], dt.float32, kind="ExternalInput")
    bvc_d = nc.dram_tensor("bvc", [P, DC], dt.float32, kind="ExternalInput")
    bob_d = nc.dram_tensor("bob", [P, DC], dt.float32, kind="ExternalInput")
    bvr_d = nc.dram_tensor("bvr", [1, D], dt.bfloat16, kind="ExternalInput")
    one8_d = nc.dram_tensor("one8", [P, 2, 16], dt.float8e4, kind="ExternalInput")
    yT_d = nc.dram_tensor("yT", [P, DC, T], dt.bfloat16, kind="ExternalOutput")
    if debug:
        dbg = {
            "sq": nc.dram_tensor("dbg_sq", [P, DC, T], dt.float8e4, kind="ExternalOutput"),
            "sk": nc.dram_tensor("dbg_sk", [P, NG, 2, D], dt.float8e4, kind="ExternalOutput"),
            "gt": nc.dram_tensor("dbg_gt", [P, DC, D], dt.bfloat16, kind="ExternalOutput"),
            "kv": nc.dram_tensor("dbg_kv", [P, DC, P], dt.bfloat16, kind="ExternalOutput"),
            "m8": nc.dram_tensor("dbg_m8", [P, DC, D], dt.float8e4, kind="ExternalOutput"),
            "csx": nc.dram_tensor("dbg_csx", [P, DC], dt.bfloat16, kind="ExternalOutput"),
            "cv": nc.dram_tensor("dbg_cv", [P, DC], dt.float32, kind="ExternalOutput"),
            "rs": nc.dram_tensor("dbg_rs", [1, D], dt.bfloat16, kind="ExternalOutput"),
            "by": nc.dram_tensor("dbg_by", [P, DC], dt.float32, kind="ExternalOutput"),
        }

    with TileContext(nc) as tc:
        with tc.tile_pool(name="persist", bufs=1) as pp:
            bqs_sb = pp.tile([P, DC], dt.float32, tag="bqs")
            bqa_sb = pp.tile([P, DC], dt.float32, tag="bqa")
            bvc_sb = pp.tile([P, DC], dt.float32, tag="bvc")
            bob_sb = pp.tile([P, DC], dt.float32, tag="bob")
            bvr_sb = pp.tile([1, D], dt.bfloat16, tag="bvr")
            one8_sb = pp.tile([P, 2, 16], dt.float8e4, tag="one8")
            zz = pp.tile([1, 640], dt.bfloat16, tag="zz")
            csxb_sb = pp.tile([P, DC], dt.bfloat16, tag="csxb")
            cv_sb = pp.tile([P, DC], dt.float32, tag="cv")
            u_sb = pp.tile([P, DC], dt.float32, tag="u")
            ub_sb = pp.tile([P, DC], dt.bfloat16, tag="ub")
            by_sb = pp.tile([P, DC], dt.float32, tag="by")
            rs_sb = pp.tile([1, D], dt.bfloat16, tag="rs")
            kvch = pp.tile([P, DC, P], dt.bfloat16, tag="kvch")
            m8_sb = pp.tile([P, DC, D], dt.float8e4, tag="m8")
            sq_sb = pp.tile([P, DC, T], dt.float8e4, tag="sq")

            nc.vector.memset(zz[:], 0.0)
            nc.vector.memset(kvch[:], 0.0)

            with tc.tile_pool(name="bigB", bufs=1) as pb:
                sk_sb = pb.tile([P, NG, 2, D], dt.float8e4, tag="sk")
                xt8_sb = pb.tile([P, NG, 2, D], dt.float8e4, tag="xt8")
                gt_sb = pb.tile([P, DC, D], dt.bfloat16, tag="gt")

                with tc.tile_pool(name="ph1w", bufs=1) as pc:
                    wq8_sb = pc.tile([P, DC, D], dt.float8e4, tag="wq8")
                    wk8_sb = pc.tile([P, DC, D], dt.float8e4, tag="wk8")
                    xT8_sb = pc.tile([P, DC, T], dt.float8e4, tag="xT8")
                    xr8_sb = pc.tile([P, NG, 2, D], dt.float8e4, tag="xr8")

                    # one queue (gpsimd: 25ns/trigger), strict priority order
                    nc.gpsimd.dma_start(wk8_sb[:], wk8_d[:])
                    nc.gpsimd.dma_start(xT8_sb[:, :, 0:1024], xT8_d[:, :, 0:1024])
                    nc.gpsimd.dma_start(xT8_sb[:, :, 1024:2048], xT8_d[:, :, 1024:2048])
                    nc.gpsimd.dma_start(wq8_sb[:], wq8_d[:])
                    nc.gpsimd.dma_start(xT8_sb[:, :, 2048:3072], xT8_d[:, :, 2048:3072])
                    nc.gpsimd.dma_start(xT8_sb[:, :, 3072:4096], xT8_d[:, :, 3072:4096])
                    nc.gpsimd.dma_start(bqs_sb[:], bqs_d[:])
                    nc.gpsimd.dma_start(bqa_sb[:], bqa_d[:])
                    nc.gpsimd.dma_start(one8_sb[:], one8_d[:])
                    nc.gpsimd.dma_start(bvc_sb[:], bvc_d[:])
                    nc.gpsimd.dma_start(bob_sb[:], bob_d[:])
                    nc.gpsimd.dma_start(bvr_sb[:], bvr_d[:])
                    for qq in range(4):
                        nc.gpsimd.dma_start(
                            xt8_sb[:, qq * 4 : (qq + 1) * 4, :, :],
                            xt8_d[:, qq * 4 : (qq + 1) * 4, :, :],
                        )
                    for hh in range(2):
                        nc.gpsimd.dma_start(
                            xr8_sb[:, hh * 8 : (hh + 1) * 8, :, :],
                            xr8_d[:, hh * 8 : (hh + 1) * 8, :, :],
                        )

                    # ---- phase 1a: k projection only (deep kps pipeline) ----
                    with tc.tile_pool(name="kps", bufs=4, space="PSUM") as kpool:
                        warm = kpool.tile([P, D], dt.float32, tag="kp")
                        for w in range(80):
                            nc.tensor.matmul(
                                warm[:, 0:128], lhsT=zz[:1, :P],
                                rhs=zz[:1, P : P + 128],
                                start=True, stop=True, skip_group_check=True,
                            )

                        def _k_group(g):
                            for i in range(2):
                                kp = kpool.tile([P, D], dt.float32, tag="kp")
                                t0 = g * 256 + i * 128
                                for ds in range(4):
                                    for c in range(4):
                                        nc.tensor.matmul(
                                            kp[:, ds * 256 : (ds + 1) * 256],
                                            lhsT=xT8_sb[:, 2 * c : 2 * c + 2, t0 : t0 + 128],
                                            rhs=wk8_sb[:, 2 * c : 2 * c + 2, ds * 256 : (ds + 1) * 256],
                                            start=(c == 0), stop=(c == 3),
                                            perf_mode=DR, skip_group_check=True,
                                        )
                                nc.scalar.activation(
                                    sk_sb[:, g, i, :], kp[:], AF.Silu,
                                    scale=SCALE / WS,
                                )

                        for g in range(NG):
                            _k_group(g)

                    # ---- phase 1b: q proj interleaved with G half-chunks ----
                    with (
                        tc.tile_pool(name="qps", bufs=4, space="PSUM") as qpool,
                        tc.tile_pool(name="gps", bufs=2, space="PSUM") as gpool,
                    ):
                        def _q_group(g):
                            for half in range(4):
                                oc0 = half * 2
                                qp = qpool.tile([P, 2, 256], dt.float32, tag="qp")
                                for j in range(2):
                                    oc = oc0 + j
                                    for c in range(4):
                                        nc.tensor.matmul(
                                            qp[:, j, :],
                                            lhsT=wq8_sb[:, 2 * c : 2 * c + 2, oc * P : (oc + 1) * P],
                                            rhs=xT8_sb[:, 2 * c : 2 * c + 2, g * 256 : (g + 1) * 256],
                                            start=(c == 0), stop=(c == 3),
                                            perf_mode=DR, skip_group_check=True,
                                        )
                                dst = sq_sb[:, oc0 : oc0 + 2, g * 256 : (g + 1) * 256]
                                if half % 2 == 0:
                                    # bias fused into ACT silu, per oc
                                    for j in range(2):
                                        nc.scalar.activation(
                                            sq_sb[:, oc0 + j, g * 256 : (g + 1) * 256],
                                            qp[:, j, :], AF.Silu,
                                            bias=bqa_sb[:, oc0 + j : oc0 + j + 1],
                                            scale=SCALE / WS,
                                        )
                                else:
                                    # bias on DVE (pre-scaled by WS host-side),
                                    # plain silu on ACT
                                    for j in range(2):
                                        nc.vector.tensor_scalar_add(
                                            qp[:, j, :], qp[:, j, :],
                                            bqs_sb[:, oc0 + j : oc0 + j + 1],
                                        )
                                    nc.scalar.activation(
                                        dst, qp[:], AF.Silu, scale=SCALE / WS,
                                    )

                        g_tiles = {}

                        def _g_half(idx):
                            cc, half = idx // 2, idx % 2
                            if half == 0:
                                gp = gpool.tile([P, D], dt.float32, tag="gp")
                                g_tiles[cc] = gp
                                # pre-zero so interleaved 256-col regions can
                                # accumulate start=False
                                for hh in range(2):
                                    nc.tensor.matmul(
                                        gp[:, hh * 512 : (hh + 1) * 512],
                                        lhsT=zz[:1, :P], rhs=zz[:1, P : P + 512],
                                        start=True, stop=True, skip_group_check=True,
                                    )
                            else:
                                gp = g_tiles[cc]
                            for g in range(half * 8, half * 8 + 8):
                                for ds in range(4):
                                    nc.tensor.matmul(
                                        gp[:, ds * 256 : (ds + 1) * 256],
                                        lhsT=xt8_sb[:, g, :, cc * P : (cc + 1) * P],
                                        rhs=sk_sb[:, g, :, ds * 256 : (ds + 1) * 256],
                                        start=False, stop=(g == NG - 1),
                                        perf_mode=DR, skip_group_check=True,
                                    )
                            if half == 1:
                                nc.vector.tensor_copy(out=gt_sb[:, cc, :], in_=gp[:])

                        for g in range(NG):
                            _q_group(g)
                            _g_half(g)

                    # ---- rowsum(s_k) + colsum(x) (PE ones-matmuls) ----
                    with tc.tile_pool(name="rscs", bufs=1, space="PSUM") as rcpool:
                        rsp = rcpool.tile([1, D], dt.float32, tag="rsp")
                        csp = rcpool.tile([P, DC], dt.float32, tag="csp")
                        for ds in range(4):
                            for g in range(NG):
                                nc.tensor.matmul(
                                    rsp[:, ds * 256 : (ds + 1) * 256],
                                    lhsT=one8_sb[:, :, 0:1],
                                    rhs=sk_sb[:, g, :, ds * 256 : (ds + 1) * 256],
                                    start=(g == 0), stop=(g == NG - 1),
                                    perf_mode=DR, skip_group_check=True,
                                )
                        nc.scalar.copy(out=rs_sb[:], in_=rsp[:])
                        # colsum_x column: contract tokens against ones;
                        # x8 and the fp8 residual accumulate into one region
                        for cc in range(DC):
                            for g in range(NG):
                                nc.tensor.matmul(
                                    csp[:, cc : cc + 1],
                                    lhsT=xt8_sb[:, g, :, cc * P : (cc + 1) * P],
                                    rhs=one8_sb[:, :, 0:1],
                                    start=(g == 0), stop=False,
                                    perf_mode=DR, skip_group_check=True,
                                )
                            for g in range(NG):
                                nc.tensor.matmul(
                                    csp[:, cc : cc + 1],
                                    lhsT=xr8_sb[:, g, :, cc * P : (cc + 1) * P],
                                    rhs=one8_sb[:, :, 0:1],
                                    start=False, stop=(g == NG - 1),
                                    perf_mode=DR, skip_group_check=True,
                                )
                        nc.vector.tensor_copy(out=csxb_sb[:], in_=csp[:])

                # ---------------- cv / kv / M / cm ----------------
                with tc.tile_pool(name="postw", bufs=1) as pd:
                    wvT_sb = pd.tile([P, DC, D], dt.bfloat16, tag="wvT")
                    woT_sb = pd.tile([P, DC, D], dt.bfloat16, tag="woT")
                    nc.sync.dma_start(wvT_sb[:], wvT_d[:])
                    nc.sync.dma_start(woT_sb[:], woT_d[:])

                    with tc.tile_pool(name="cvps", bufs=1, space="PSUM") as cvpool:
                        cvp = cvpool.tile([P, DC], dt.float32, tag="cvp")
                        # colsum_v = Wv @ colsum_x  (+ T*bv via bvc)
                        for b in range(DC):
                            for cc in range(DC):
                                nc.tensor.matmul(
                                    cvp[:, b : b + 1],
                                    lhsT=wvT_sb[:, cc, b * P : (b + 1) * P],
                                    rhs=csxb_sb[:, cc : cc + 1],
                                    start=(cc == 0), stop=(cc == DC - 1),
                                    skip_group_check=True,
                                )
                        nc.vector.tensor_add(cv_sb[:], cvp[:], bvc_sb[:])

                    # kv blocks
                    with tc.tile_pool(name="kvps", bufs=2, space="PSUM") as kvpool:
                        for half in range(2):
                            kvp = kvpool.tile([P, 4, P], dt.float32, tag="kvp")
                            for j in range(4):
                                b = half * 4 + j
                                for cc in range(DC):
                                    nc.tensor.matmul(
                                        kvp[:, j, :],
                                        lhsT=wvT_sb[:, cc, b * P : (b + 1) * P],
                                        rhs=gt_sb[:, cc, b * P : (b + 1) * P],
                                        start=(cc == 0), stop=False,
                                        skip_group_check=True,
                                    )
                                nc.tensor.matmul(
                                    kvp[:, j, :],
                                    lhsT=bvr_sb[:1, b * P : (b + 1) * P],
                                    rhs=rs_sb[:1, b * P : (b + 1) * P],
                                    start=False, stop=True, skip_group_check=True,
                                )
                            for j in range(4):
                                b = half * 4 + j
                                nc.scalar.activation(
                                    kvch[0:64, b, 0:64], kvp[0:64, j, 0:64],
                                    AF.Identity, bias=cv_sb[0:64, b : b + 1],
                                )
                                nc.scalar.activation(
                                    kvch[64:128, b, 64:128], kvp[64:128, j, 64:128],
                                    AF.Identity, bias=cv_sb[64:128, b : b + 1],
                                )
                                nc.vector.tensor_reduce(
                                    u_sb[0:64, b : b + 1], kvch[0:64, b, 0:64],
                                    axis=mybir.AxisListType.X, op=mybir.AluOpType.add,
                                )
                                nc.vector.tensor_reduce(
                                    u_sb[64:128, b : b + 1], kvch[64:128, b, 64:128],
                                    axis=mybir.AxisListType.X, op=mybir.AluOpType.add,
                                )
                        nc.vector.tensor_copy(out=ub_sb[:], in_=u_sb[:])

                    # M = kv^T @ Wo^T ; colsum_M
                    with tc.tile_pool(name="mps", bufs=2, space="PSUM") as mpool:
                        for b in range(DC):
                            mp = mpool.tile([P, D], dt.float32, tag="mp")
                            for hh in range(2):
                                nc.tensor.matmul(
                                    mp[:, hh * 512 : (hh + 1) * 512],
                                    lhsT=kvch[:, b, :],
                                    rhs=woT_sb[:, b, hh * 512 : (hh + 1) * 512],
                                    start=True, stop=True, skip_group_check=True,
                                )
                            if b % 2 == 0:
                                nc.scalar.copy(out=m8_sb[:, b, :], in_=mp[:])
                            else:
                                nc.vector.tensor_copy(out=m8_sb[:, b, :], in_=mp[:])
                        cmp_t = mpool.tile([P, DC], dt.float32, tag="cmp")
                        for oc in range(DC):
                            for b in range(DC):
                                nc.tensor.matmul(
                                    cmp_t[:, oc : oc + 1],
                                    lhsT=woT_sb[:, b, oc * P : (oc + 1) * P],
                                    rhs=ub_sb[:, b : b + 1],
                                    start=(b == 0), stop=(b == DC - 1),
                                    skip_group_check=True,
                                )
                        nc.vector.tensor_add(by_sb[:], cmp_t[:], bob_sb[:])

                    if debug:
                        nc.sync.dma_start(dbg["sq"][:], sq_sb[:])
                        nc.sync.dma_start(dbg["sk"][:], sk_sb[:])
                        nc.sync.dma_start(dbg["gt"][:], gt_sb[:])
                        nc.sync.dma_start(dbg["kv"][:], kvch[:])
                        nc.sync.dma_start(dbg["m8"][:], m8_sb[:])
                        nc.sync.dma_start(dbg["csx"][:], csxb_sb[:])
                        nc.sync.dma_start(dbg["cv"][:], cv_sb[:])
                        nc.sync.dma_start(dbg["rs"][:], rs_sb[:])
                        nc.sync.dma_start(dbg["by"][:], by_sb[:])

            # ---------------- phase 2: y^T = M8^T s_q + bias ----------------
            with (
                tc.tile_pool(name="yout", bufs=6) as ypool,
                tc.tile_pool(name="yps", bufs=8, space="PSUM") as ypsp,
            ):
                n = 0
                for oc in range(DC):
                    for tp in range(8):
                        if tp % 2 == 0:
                            ys = ypool.tile(
                                [P, 2, 512], dt.bfloat16, tag="ys", name="ys"
                            )
                        yp = ypsp.tile([P, 512], dt.float32, tag="yp")
                        for hh in range(2):
                            ts = tp * 2 + hh
                            for f in range(4):
                                nc.tensor.matmul(
                                    yp[:, hh * 256 : (hh + 1) * 256],
                                    lhsT=m8_sb[:, 2 * f : 2 * f + 2, oc * P : (oc + 1) * P],
                                    rhs=sq_sb[:, 2 * f : 2 * f + 2, ts * 256 : (ts + 1) * 256],
                                    start=(f == 0), stop=(f == 3),
                                    perf_mode=DR, skip_group_check=True,
                                )
                        if n % 2 == 0:
                            nc.scalar.activation(
                                ys[:, tp % 2, :], yp[:], AF.Identity,
                                bias=by_sb[:, oc : oc + 1], scale=1.0,
                            )
                        else:
                            nc.vector.tensor_scalar_add(
                                ys[:, tp % 2, :], yp[:], by_sb[:, oc : oc + 1]
                            )
                        if tp % 2 == 1:
                            (nc.sync if (n // 2) % 2 == 0 else nc.gpsimd).dma_start(
                                yT_d[:, oc, (tp - 1) * 512 : (tp + 1) * 512],
                                ys[:],
                            )
                        n += 1

    _split_multi_waits(nc)
    return nc


def _get_program(debug=False):
    key = ("nc", debug)
    if key not in _CACHE:
        _CACHE[key] = _build_program(debug)
    return _CACHE[key]


def _prep_shared(Wq, bq, Wk, Wv, bv, Wo, bo):
    def wchunk(w, dtype, scale=1.0):
        # [D, D] row-major (contract, out) -> [P, DC, D] with c = cc*128+p
        return np.ascontiguousarray(
            (w * scale).T.reshape(DC, P, D).transpose(1, 0, 2)
        ).astype(dtype)

    shared = {
        "wq8": wchunk(Wq, _F8, WS),
        "wk8": wchunk(Wk, _F8, WS),
        "wvT": wchunk(Wv, _BF16),
        "woT": wchunk(Wo, _BF16),
        # DVE/ACT pre-add this to the WS-scaled q PSUM; ACT then multiplies
        # by SCALE/WS, so the bias carries WS (not SCALE).
        "bqs": np.ascontiguousarray((WS * bq).astype(np.float32).reshape(DC, P).T),
        "bqa": np.ascontiguousarray((SCALE * bq).astype(np.float32).reshape(DC, P).T),
        "bvc": np.ascontiguousarray((T * bv).astype(np.float32).reshape(DC, P).T),
        "bob": np.ascontiguousarray(bo.astype(np.float32).reshape(DC, P).T),
        "bvr": bv.astype(_BF16)[None, :],
        "one8": np.ones((P, 2, 16), _F8),
    }
    return shared


def _prep_x(xb):
    xT = np.ascontiguousarray(xb.T)  # [D, T]
    x8 = xb.astype(_F8)
    xr8 = (xb - x8.astype(np.float32)).astype(_F8)

    def tok(a):
        return np.ascontiguousarray(
            a.reshape(NG, 2, P, D).transpose(2, 0, 1, 3)
        )

    return {
        "xT8": np.ascontiguousarray(
            xT.reshape(DC, P, T).transpose(1, 0, 2)
        ).astype(_F8),
        "xt8": tok(x8),
        "xr8": tok(xr8),
    }


def _run(in_maps, trace=False, debug=False, **kw):
    from concourse.bass_utils import run_bass_kernel_spmd

    nc = _get_program(debug)
    return run_bass_kernel_spmd(nc, in_maps, list(range(len(in_maps))), trace=trace, **kw)


def kernel(x, Wq, bq, Wk, Wv, bv, Wo, bo):
    x = np.asarray(x, dtype=np.float32)
    assert x.shape == (B, T, D), x.shape
    shared = _prep_shared(
        np.asarray(Wq, np.float32), np.asarray(bq, np.float32),
        np.asarray(Wk, np.float32), np.asarray(Wv, np.float32),
        np.asarray(bv, np.float32), np.asarray(Wo, np.float32),
        np.asarray(bo, np.float32),
    )
    in_maps = []
    for b in range(B):
        m = dict(shared)
        m.update(_prep_x(x[b]))
        in_maps.append(m)

    res = _run(in_maps)
    out = np.empty((B, T, D), np.float32)
    for b in range(B):
        yT = np.asarray(res.results[b]["yT"]).astype(np.float32)  # [P, DC, T]
        out[b] = yT.transpose(1, 0, 2).reshape(D, T).T
    return out


# revision 19
# speedup vs baseline: 2.9759x; 1.0511x over previous
"""Linear attention (silu+1 feature map) MultiHeadAttention for 8x TRN2.

Sharding: data-parallel over batch (B=8 -> 1 batch element per NeuronCore).

Math per core (T=4096, D=1024, H=16, Dh=64), with phi(z) = 1 + s(z),
s(z) = silu(z). Write s_q = silu(scale*q), s_k = silu(scale*k). Then

  kv_h   = phi_k_h^T v_h
         = colsum_v_h                      (rank-1 in e; exact, bf16/fp32)
         + (s_k^T x)_h @ Wv_h^T           (fp8 "G path": replaces v proj)
         + bv_h (x) rowsum(s_k)_h         (rank-1 correction)
  M      = kv^T-blocks @ Wo^T             (block-diag, bf16)
  y^T    = M8^T @ s_q + colsum_M + bo     (fp8; the +1 of phi_q is folded
                                           into colsum_M = ones^T M)

All big GEMMs (q proj, k proj, G = s_k^T x, phase-2) run as fp8-e4m3
DoubleRow matmuls (2x128-row contraction @ 0.5 cyc/row).  Centering the
+1 out of phi keeps fp8 noise confined to the ~12%-magnitude fluctuation
terms; exact colsums are carried in fp32/bf16.  Weights are scaled by 64
before fp8 quantization to clear the e4m3 subnormal floor; the inverse
scale rides the ACT silu drain.
"""

import numpy as np
import ml_dtypes

B, T, D = 8, 4096, 1024
H, DH = 16, 64
SCALE = float(DH ** -0.25)
NCORES = 8
P = 128
DC = D // P            # 8 feature chunks
NG = T // 256          # 16 groups of 256 tokens
WS = 64.0              # fp8 weight prescale

_BF16 = ml_dtypes.bfloat16
_F8 = ml_dtypes.float8_e4m3

_CACHE = {}


def _split_multi_waits(nc):
    """walrus in this container only encodes ONE sync-wait command per
    instruction. Hoist extra waits onto injected same-engine NOPs placed
    immediately before the instruction."""
    import concourse.mybir as mybir

    n_split = 0
    for fn in nc.m.functions:
        for bb in fn.blocks:
            new = []
            changed = False
            for inst in bb.instructions:
                si = inst.sync_info
                waits = list(si.on_wait) if si is not None else []
                if len(waits) > 1:
                    changed = True
                    for j, w in enumerate(waits[:-1]):
                        nop = mybir.InstNoOp(
                            name=f"{inst.name}-sw{j}", ins=[], outs=[]
                        )
                        nop.engine = inst.engine
                        nop.sync_info = mybir.SyncInfo(
                            on_wait=[w], on_update=[]
                        )
                        new.append(nop)
                        n_split += 1
                    inst.sync_info = mybir.SyncInfo(
                        on_wait=[waits[-1]], on_update=list(si.on_update)
                    )
                new.append(inst)
            if changed:
                bb.instructions = new
    return n_split


def _build_program(debug=False):
    import concourse.bass as bass
    import concourse.mybir as mybir
    from concourse.tile import TileContext

    dt = mybir.dt
    AF = mybir.ActivationFunctionType
    DR = mybir.MatmulPerfMode.DoubleRow

    nc = bass.Bass()

    xT8_d = nc.dram_tensor("xT8", [P, DC, T], dt.float8e4, kind="ExternalInput")
    xt8_d = nc.dram_tensor("xt8", [P, NG, 2, D], dt.float8e4, kind="ExternalInput")
    xr8_d = nc.dram_tensor("xr8", [P, NG, 2, D], dt.float8e4, kind="ExternalInput")
    wq8_d = nc.dram_tensor("wq8", [P, DC, D], dt.float8e4, kind="ExternalInput")
    wk8_d = nc.dram_tensor("wk8", [P, DC, D], dt.float8e4, kind="ExternalInput")
    wvT_d = nc.dram_tensor("wvT", [P, DC, D], dt.bfloat16, kind="ExternalInput")
    woT_d = nc.dram_tensor("woT", [P, DC, D], dt.bfloat16, kind="ExternalInput")
    bqs_d = nc.dram_tensor("bqs", [P, DC], dt.float32, kind="ExternalInput")
    bqa_d = nc.dram_tensor("bqa", [P, DC], dt.float32, kind="ExternalInput")
    bvc_d = nc.dram_tensor("bvc", [P, DC], dt.float32, kind="ExternalInput")
    bob_d = nc.dram_tensor("bob", [P, DC], dt.float32, kind="ExternalInput")
    bvr_d = nc.dram_tensor("bvr", [1, D], dt.bfloat16, kind="ExternalInput")
    one8_d = nc.dram_tensor("one8", [P, 2, 16], dt.float8e4, kind="ExternalInput")
    yT_d = nc.dram_tensor("yT", [P, DC, T], dt.bfloat16, kind="ExternalOutput")
    if debug:
        dbg = {
            "sq": nc.dram_tensor("dbg_sq", [P, DC, T], dt.float8e4, kind="ExternalOutput"),
            "sk": nc.dram_tensor("dbg_sk", [P, NG, 2, D], dt.float8e4, kind="ExternalOutput"),
            "gt": nc.dram_tensor("dbg_gt", [P, DC, D], dt.bfloat16, kind="ExternalOutput"),
            "kv": nc.dram_tensor("dbg_kv", [P, DC, P], dt.bfloat16, kind="ExternalOutput"),
            "m8": nc.dram_tensor("dbg_m8", [P, DC, D], dt.float8e4, kind="ExternalOutput"),
            "csx": nc.dram_tensor("dbg_csx", [P, DC], dt.bfloat16, kind="ExternalOutput"),
            "cv": nc.dram_tensor("dbg_cv", [P, DC], dt.float32, kind="ExternalOutput"),
            "rs": nc.dram_tensor("dbg_rs", [1, D], dt.bfloat16, kind="ExternalOutput"),
            "by": nc.dram_tensor("dbg_by", [P, DC], dt.float32, kind="ExternalOutput"),
        }

    with TileContext(nc) as tc:
      with tc.tile_pool(name="persist", bufs=1) as pp:
        bqs_sb = pp.tile([P, DC], dt.float32, tag="bqs")
        bqa_sb = pp.tile([P, DC], dt.float32, tag="bqa")
        bvc_sb = pp.tile([P, DC], dt.float32, tag="bvc")
        bob_sb = pp.tile([P, DC], dt.float32, tag="bob")
        bvr_sb = pp.tile([1, D], dt.bfloat16, tag="bvr")
        one8_sb = pp.tile([P, 2, 16], dt.float8e4, tag="one8")
        zz = pp.tile([1, 640], dt.bfloat16, tag="zz")
        csxb_sb = pp.tile([P, DC], dt.bfloat16, tag="csxb")
        cv_sb = pp.tile([P, DC], dt.float32, tag="cv")
        u_sb = pp.tile([P, DC], dt.float32, tag="u")
        ub_sb = pp.tile([P, DC], dt.bfloat16, tag="ub")
        by_sb = pp.tile([P, DC], dt.float32, tag="by")
        rs_sb = pp.tile([1, D], dt.bfloat16, tag="rs")
        kvch = pp.tile([P, DC, P], dt.bfloat16, tag="kvch")
        m8_sb = pp.tile([P, DC, D], dt.float8e4, tag="m8")
        sq_sb = pp.tile([P, DC, T], dt.float8e4, tag="sq")

        nc.vector.memset(zz[:], 0.0)
        nc.vector.memset(kvch[:], 0.0)

        with tc.tile_pool(name="bigB", bufs=1) as pb:
          sk_sb = pb.tile([P, NG, 2, D], dt.float8e4, tag="sk")
          xt8_sb = pb.tile([P, NG, 2, D], dt.float8e4, tag="xt8")
          gt_sb = pb.tile([P, DC, D], dt.bfloat16, tag="gt")

          with tc.tile_pool(name="ph1w", bufs=1) as pc:
            wq8_sb = pc.tile([P, DC, D], dt.float8e4, tag="wq8")
            wk8_sb = pc.tile([P, DC, D], dt.float8e4, tag="wk8")
            xT8_sb = pc.tile([P, DC, T], dt.float8e4, tag="xT8")

            # ============ phase 1a (k proj) + colsum-x, xr8 scoped ============
            with tc.tile_pool(name="ph1r", bufs=1) as pcr:
                xr8_sb = pcr.tile([P, NG, 2, D], dt.float8e4, tag="xr8")

                # one DMA queue (gpsimd: 25ns/trigger), strict priority order
                nc.gpsimd.dma_start(wk8_sb[:], wk8_d[:])
                nc.gpsimd.dma_start(xT8_sb[:, :, 0:1024], xT8_d[:, :, 0:1024])
                nc.gpsimd.dma_start(xT8_sb[:, :, 1024:2048], xT8_d[:, :, 1024:2048])
                nc.gpsimd.dma_start(wq8_sb[:], wq8_d[:])
                nc.gpsimd.dma_start(xT8_sb[:, :, 2048:3072], xT8_d[:, :, 2048:3072])
                nc.gpsimd.dma_start(xT8_sb[:, :, 3072:4096], xT8_d[:, :, 3072:4096])
                nc.gpsimd.dma_start(bqs_sb[:], bqs_d[:])
                nc.gpsimd.dma_start(bqa_sb[:], bqa_d[:])
                nc.gpsimd.dma_start(one8_sb[:], one8_d[:])
                nc.gpsimd.dma_start(bvc_sb[:], bvc_d[:])
                nc.gpsimd.dma_start(bob_sb[:], bob_d[:])
                nc.gpsimd.dma_start(bvr_sb[:], bvr_d[:])
                for qq in range(4):
                    nc.gpsimd.dma_start(
                        xt8_sb[:, qq * 4 : (qq + 1) * 4, :, :],
                        xt8_d[:, qq * 4 : (qq + 1) * 4, :, :],
                    )
                for hh in range(2):
                    nc.gpsimd.dma_start(
                        xr8_sb[:, hh * 8 : (hh + 1) * 8, :, :],
                        xr8_d[:, hh * 8 : (hh + 1) * 8, :, :],
                    )

                with (
                    tc.tile_pool(name="kps", bufs=3, space="PSUM") as kpool,
                    tc.tile_pool(name="csps", bufs=1, space="PSUM") as cspool,
                ):
                    csp = cspool.tile([P, DC], dt.float32, tag="csp")
                    warm = kpool.tile([P, D], dt.float32, tag="kp")
                    for w in range(100):
                        nc.tensor.matmul(
                            warm[:, 0:128], lhsT=zz[:1, :P],
                            rhs=zz[:1, P : P + 128],
                            start=True, stop=True, skip_group_check=True,
                        )

                    def _k_group(g):
                        for i in range(2):
                            kp = kpool.tile([P, D], dt.float32, tag="kp")
                            t0 = g * 256 + i * 128
                            for ds in range(4):
                                for c in range(4):
                                    nc.tensor.matmul(
                                        kp[:, ds * 256 : (ds + 1) * 256],
                                        lhsT=xT8_sb[:, 2 * c : 2 * c + 2, t0 : t0 + 128],
                                        rhs=wk8_sb[:, 2 * c : 2 * c + 2, ds * 256 : (ds + 1) * 256],
                                        start=(c == 0), stop=(c == 3),
                                        perf_mode=DR, skip_group_check=True,
                                    )
                            nc.scalar.activation(
                                sk_sb[:, g, i, :], kp[:], AF.Silu,
                                scale=SCALE / WS,
                            )

                    for g in range(NG):
                        _k_group(g)

                    # colsum_x column: contract tokens against ones; x8 and
                    # the fp8 residual accumulate into one region
                    for cc in range(DC):
                        for g in range(NG):
                            nc.tensor.matmul(
                                csp[:, cc : cc + 1],
                                lhsT=xt8_sb[:, g, :, cc * P : (cc + 1) * P],
                                rhs=one8_sb[:, :, 0:1],
                                start=(g == 0), stop=False,
                                perf_mode=DR, skip_group_check=True,
                            )
                        for g in range(NG):
                            nc.tensor.matmul(
                                csp[:, cc : cc + 1],
                                lhsT=xr8_sb[:, g, :, cc * P : (cc + 1) * P],
                                rhs=one8_sb[:, :, 0:1],
                                start=False, stop=(g == NG - 1),
                                perf_mode=DR, skip_group_check=True,
                            )
                    nc.vector.tensor_copy(out=csxb_sb[:], in_=csp[:])

            # ====== xr8 space free: load wvT/woT during 1b ======
            with tc.tile_pool(name="postw", bufs=1) as pd:
                wvT_sb = pd.tile([P, DC, D], dt.bfloat16, tag="wvT")
                woT_sb = pd.tile([P, DC, D], dt.bfloat16, tag="woT")
                nc.sync.dma_start(wvT_sb[:], wvT_d[:])
                nc.sync.dma_start(woT_sb[:], woT_d[:])

                # ====== phase 1b: q proj interleaved with G half-chunks ======
                with (
                    tc.tile_pool(name="qps", bufs=4, space="PSUM") as qpool,
                    tc.tile_pool(name="gps", bufs=2, space="PSUM") as gpool,
                ):
                    def _q_group(g):
                        for half in range(4):
                            oc0 = half * 2
                            qp = qpool.tile([P, 2, 256], dt.float32, tag="qp")
                            for j in range(2):
                                oc = oc0 + j
                                for c in range(4):
                                    nc.tensor.matmul(
                                        qp[:, j, :],
                                        lhsT=wq8_sb[:, 2 * c : 2 * c + 2, oc * P : (oc + 1) * P],
                                        rhs=xT8_sb[:, 2 * c : 2 * c + 2, g * 256 : (g + 1) * 256],
                                        start=(c == 0), stop=(c == 3),
                                        perf_mode=DR, skip_group_check=True,
                                    )
                            dst = sq_sb[:, oc0 : oc0 + 2, g * 256 : (g + 1) * 256]
                            if half % 2 == 0:
                                # bias fused into ACT silu, per oc
                                for j in range(2):
                                    nc.scalar.activation(
                                        sq_sb[:, oc0 + j, g * 256 : (g + 1) * 256],
                                        qp[:, j, :], AF.Silu,
                                        bias=bqa_sb[:, oc0 + j : oc0 + j + 1],
                                        scale=SCALE / WS,
                                    )
                            else:
                                # bias on DVE (WS-scaled), plain silu on ACT
                                for j in range(2):
                                    nc.vector.tensor_scalar_add(
                                        qp[:, j, :], qp[:, j, :],
                                        bqs_sb[:, oc0 + j : oc0 + j + 1],
                                    )
                                nc.scalar.activation(
                                    dst, qp[:], AF.Silu, scale=SCALE / WS,
                                )

                    g_tiles = {}

                    def _g_half(idx):
                        cc, half = idx // 2, idx % 2
                        if half == 0:
                            gp = gpool.tile([P, D], dt.float32, tag="gp")
                            g_tiles[cc] = gp
                            # pre-zero so interleaved 256-col regions can
                            # accumulate start=False
                            for hh in range(2):
                                nc.tensor.matmul(
                                    gp[:, hh * 512 : (hh + 1) * 512],
                                    lhsT=zz[:1, :P], rhs=zz[:1, P : P + 512],
                                    start=True, stop=True, skip_group_check=True,
                                )
                        else:
                            gp = g_tiles[cc]
                        for g in range(half * 8, half * 8 + 8):
                            for ds in range(4):
                                nc.tensor.matmul(
                                    gp[:, ds * 256 : (ds + 1) * 256],
                                    lhsT=xt8_sb[:, g, :, cc * P : (cc + 1) * P],
                                    rhs=sk_sb[:, g, :, ds * 256 : (ds + 1) * 256],
                                    start=False, stop=(g == NG - 1),
                                    perf_mode=DR, skip_group_check=True,
                                )
                        if half == 1:
                            nc.vector.tensor_copy(out=gt_sb[:, cc, :], in_=gp[:])

                    for g in range(NG):
                        _q_group(g)
                        _g_half(g)

                # ---- rowsum(s_k) (PE ones-matmuls) ----
                with tc.tile_pool(name="rscs", bufs=1, space="PSUM") as rcpool:
                    rsp = rcpool.tile([1, D], dt.float32, tag="rsp")
                    cvp = rcpool.tile([P, DC], dt.float32, tag="cvp")
                    for ds in range(4):
                        for g in range(NG):
                            nc.tensor.matmul(
                                rsp[:, ds * 256 : (ds + 1) * 256],
                                lhsT=one8_sb[:, :, 0:1],
                                rhs=sk_sb[:, g, :, ds * 256 : (ds + 1) * 256],
                                start=(g == 0), stop=(g == NG - 1),
                                perf_mode=DR, skip_group_check=True,
                            )
                    nc.scalar.copy(out=rs_sb[:], in_=rsp[:])
                    # colsum_v = Wv @ colsum_x  (+ T*bv via bvc)
                    for b in range(DC):
                        for cc in range(DC):
                            nc.tensor.matmul(
                                cvp[:, b : b + 1],
                                lhsT=wvT_sb[:, cc, b * P : (b + 1) * P],
                                rhs=csxb_sb[:, cc : cc + 1],
                                start=(cc == 0), stop=(cc == DC - 1),
                                skip_group_check=True,
                            )
                    nc.vector.tensor_add(cv_sb[:], cvp[:], bvc_sb[:])

                # kv blocks
                with tc.tile_pool(name="kvps", bufs=2, space="PSUM") as kvpool:
                    for half in range(2):
                        kvp = kvpool.tile([P, 4, P], dt.float32, tag="kvp")
                        for j in range(4):
                            b = half * 4 + j
                            for cc in range(DC):
                                nc.tensor.matmul(
                                    kvp[:, j, :],
                                    lhsT=wvT_sb[:, cc, b * P : (b + 1) * P],
                                    rhs=gt_sb[:, cc, b * P : (b + 1) * P],
                                    start=(cc == 0), stop=False,
                                    skip_group_check=True,
                                )
                            nc.tensor.matmul(
                                kvp[:, j, :],
                                lhsT=bvr_sb[:1, b * P : (b + 1) * P],
                                rhs=rs_sb[:1, b * P : (b + 1) * P],
                                start=False, stop=True, skip_group_check=True,
                            )
                        for j in range(4):
                            b = half * 4 + j
                            nc.scalar.activation(
                                kvch[0:64, b, 0:64], kvp[0:64, j, 0:64],
                                AF.Identity, bias=cv_sb[0:64, b : b + 1],
                            )
                            nc.scalar.activation(
                                kvch[64:128, b, 64:128], kvp[64:128, j, 64:128],
                                AF.Identity, bias=cv_sb[64:128, b : b + 1],
                            )
                            nc.vector.tensor_reduce(
                                u_sb[0:64, b : b + 1], kvch[0:64, b, 0:64],
                                axis=mybir.AxisListType.X, op=mybir.AluOpType.add,
                            )
                            nc.vector.tensor_reduce(
                                u_sb[64:128, b : b + 1], kvch[64:128, b, 64:128],
                                axis=mybir.AxisListType.X, op=mybir.AluOpType.add,
                            )
                    nc.vector.tensor_copy(out=ub_sb[:], in_=u_sb[:])

                # M = kv^T @ Wo^T ; colsum_M
                with tc.tile_pool(name="mps", bufs=2, space="PSUM") as mpool:
                    for b in range(DC):
                        mp = mpool.tile([P, D], dt.float32, tag="mp")
                        for hh in range(2):
                            nc.tensor.matmul(
                                mp[:, hh * 512 : (hh + 1) * 512],
                                lhsT=kvch[:, b, :],
                                rhs=woT_sb[:, b, hh * 512 : (hh + 1) * 512],
                                start=True, stop=True, skip_group_check=True,
                            )
                        if b % 2 == 0:
                            nc.scalar.copy(out=m8_sb[:, b, :], in_=mp[:])
                        else:
                            nc.vector.tensor_copy(out=m8_sb[:, b, :], in_=mp[:])
                    cmp_t = mpool.tile([P, DC], dt.float32, tag="cmp")
                    for oc in range(DC):
                        for b in range(DC):
                            nc.tensor.matmul(
                                cmp_t[:, oc : oc + 1],
                                lhsT=woT_sb[:, b, oc * P : (oc + 1) * P],
                                rhs=ub_sb[:, b : b + 1],
                                start=(b == 0), stop=(b == DC - 1),
                                skip_group_check=True,
                            )
                    nc.vector.tensor_add(by_sb[:], cmp_t[:], bob_sb[:])

                if debug:
                    nc.sync.dma_start(dbg["sq"][:], sq_sb[:])
                    nc.sync.dma_start(dbg["sk"][:], sk_sb[:])
                    nc.sync.dma_start(dbg["gt"][:], gt_sb[:])
                    nc.sync.dma_start(dbg["kv"][:], kvch[:])
                    nc.sync.dma_start(dbg["m8"][:], m8_sb[:])
                    nc.sync.dma_start(dbg["csx"][:], csxb_sb[:])
                    nc.sync.dma_start(dbg["cv"][:], cv_sb[:])
                    nc.sync.dma_start(dbg["rs"][:], rs_sb[:])
                    nc.sync.dma_start(dbg["by"][:], by_sb[:])

        # ================= phase 2: y^T = M8^T s_q + bias =================
        with (
            tc.tile_pool(name="yout", bufs=6) as ypool,
            tc.tile_pool(name="yps", bufs=8, space="PSUM") as ypsp,
        ):
            n = 0
            for oc in range(DC):
                for tp in range(8):
                    if tp % 2 == 0:
                        ys = ypool.tile(
                            [P, 2, 512], dt.bfloat16, tag="ys", name="ys"
                        )
                    yp = ypsp.tile([P, 512], dt.float32, tag="yp")
                    for hh in range(2):
                        ts = tp * 2 + hh
                        for f in range(4):
                            nc.tensor.matmul(
                                yp[:, hh * 256 : (hh + 1) * 256],
                                lhsT=m8_sb[:, 2 * f : 2 * f + 2, oc * P : (oc + 1) * P],
                                rhs=sq_sb[:, 2 * f : 2 * f + 2, ts * 256 : (ts + 1) * 256],
                                start=(f == 0), stop=(f == 3),
                                perf_mode=DR, skip_group_check=True,
                            )
                    if n % 2 == 0:
                        nc.scalar.activation(
                            ys[:, tp % 2, :], yp[:], AF.Identity,
                            bias=by_sb[:, oc : oc + 1], scale=1.0,
                        )
                    else:
                        nc.vector.tensor_scalar_add(
                            ys[:, tp % 2, :], yp[:], by_sb[:, oc : oc + 1]
                        )
                    if tp % 2 == 1:
                        # last transfers on sync (HWDGE beats Pool SWDGE at
                        # the kernel tail)
                        q = nc.sync if ((n // 2) % 2 == 0 or n >= 60) \
                            else nc.gpsimd
                        q.dma_start(
                            yT_d[:, oc, (tp - 1) * 512 : (tp + 1) * 512],
                            ys[:],
                        )
                    n += 1

    _split_multi_waits(nc)
    return nc


def _get_program(debug=False):
    key = ("nc", debug)
    if key not in _CACHE:
        _CACHE[key] = _build_program(debug)
    return _CACHE[key]


def _prep_shared(Wq, bq, Wk, Wv, bv, Wo, bo):
    def wchunk(w, dtype, scale=1.0):
        # [D, D] row-major (contract, out) -> [P, DC, D] with c = cc*128+p
        return np.ascontiguousarray(
            (w * scale).T.reshape(DC, P, D).transpose(1, 0, 2)
        ).astype(dtype)

    shared = {
        "wq8": wchunk(Wq, _F8, WS),
        "wk8": wchunk(Wk, _F8, WS),
        "wvT": wchunk(Wv, _BF16),
        "woT": wchunk(Wo, _BF16),
        # DVE/ACT pre-add this to the WS-scaled q PSUM; ACT then multiplies
        # by SCALE/WS, so the bias carries WS (not SCALE).
        "bqs": np.ascontiguousarray((WS * bq).astype(np.float32).reshape(DC, P).T),
        "bqa": np.ascontiguousarray((SCALE * bq).astype(np.float32).reshape(DC, P).T),
        "bvc": np.ascontiguousarray((T * bv).astype(np.float32).reshape(DC, P).T),
        "bob": np.ascontiguousarray(bo.astype(np.float32).reshape(DC, P).T),
        "bvr": bv.astype(_BF16)[None, :],
        "one8": np.ones((P, 2, 16), _F8),
    }
    return shared


def _prep_x(xb):
    xT = np.ascontiguousarray(xb.T)  # [D, T]
    x8 = xb.astype(_F8)
    xr8 = (xb - x8.astype(np.float32)).astype(_F8)

    def tok(a):
        return np.ascontiguousarray(
            a.reshape(NG, 2, P, D).transpose(2, 0, 1, 3)
        )

    return {
        "xT8": np.ascontiguousarray(
            xT.reshape(DC, P, T).transpose(1, 0, 2)
        ).astype(_F8),
        "xt8": tok(x8),
        "xr8": tok(xr8),
    }


def _run(in_maps, trace=False, debug=False, **kw):
    from concourse.bass_utils import run_bass_kernel_spmd

    nc = _get_program(debug)
    return run_bass_kernel_spmd(nc, in_maps, list(range(len(in_maps))), trace=trace, **kw)


def kernel(x, Wq, bq, Wk, Wv, bv, Wo, bo):
    x = np.asarray(x, dtype=np.float32)
    assert x.shape == (B, T, D), x.shape
    shared = _prep_shared(
        np.asarray(Wq, np.float32), np.asarray(bq, np.float32),
        np.asarray(Wk, np.float32), np.asarray(Wv, np.float32),
        np.asarray(bv, np.float32), np.asarray(Wo, np.float32),
        np.asarray(bo, np.float32),
    )
    in_maps = []
    for b in range(B):
        m = dict(shared)
        m.update(_prep_x(x[b]))
        in_maps.append(m)

    res = _run(in_maps)
    out = np.empty((B, T, D), np.float32)
    for b in range(B):
        yT = np.asarray(res.results[b]["yT"]).astype(np.float32)  # [P, DC, T]
        out[b] = yT.transpose(1, 0, 2).reshape(D, T).T
    return out
